# revision 62
# baseline (speedup 1.0000x reference)
"""Trainium2 Bass kernel for BiomarkerConditionedAttention.

Sharding: pure data-parallel over batch B=16 across 8 cores (2 batches/core).

v2 layout strategy (per core; "T" = feature-on-partitions):
  - all matmul operands bf16 (PSUM accum f32); error budget 2e-2 allows it.
  - tokens padded 513 -> 520 on host (zero pad): every 513-free matmul
    becomes 2x260 PSUM-bank chunks - no 1-wide companion matmuls.
    Padded KEYS are neutralized by zeroing their vn rows (v and the
    ones-column), so they contribute 0 to both context and denominator.
    Padded QUERY columns produce garbage that is sliced off at the end.
  - branch1: q/k channel-major; v token-major widened with a per-head
    ones-column so attn@v also emits the softmax denominator row.
  - branch2 cross-attn via rank-1 algebra; bio_query projection folded
    into ca_query on host (W_caq_eff = W_caq @ W_bioq).
  - qk-trick and ca-value use masked/full-then-select batching to cut
    tiny-matmul count.
  - branch3 grid_sample as dense matmul with on-device trilinear weights.
  - gate: per-token part contracts only standard_out; conditioned/dba
    slabs are per-batch row terms added via a rank-1 selector matmul.
"""

import os
import sys

sys.path.insert(0, "/opt/trn_rl_repo")

import ml_dtypes
import numpy as np

import concourse.bass as bass
import concourse.mybir as mybir
import concourse.tile as tile
from concourse import bacc, bass_utils

F32 = mybir.dt.float32
BF16 = mybir.dt.bfloat16
FP8 = mybir.dt.float8e4
DR = mybir.MatmulPerfMode.DoubleRow
AF = mybir.ActivationFunctionType
ALU = mybir.AluOpType
AX = mybir.AxisListType

B, N, C, H, M, G = 16, 513, 768, 12, 5, 8
HD = C // H  # 64
ISC = float(1.0 / np.sqrt(HD))
NCORES = 8
BPC = B // NCORES  # 2
KC = C // 128  # 6
NP = 520  # padded token count
JC = 260  # free-dim chunk (2 per 520)
TC = 104  # token chunk (partition dim), 5 per 520
NREAL = 513
VW = H * (HD + 1)  # 780
N768 = [(0, 512), (512, 256)]
NVW = [(0, 512), (512, 268)]


def build(nc: bass.Bass):
    dram = {}

    def din(name, shape, dt=BF16):
        dram[name] = nc.dram_tensor(name, list(shape), dt, kind="ExternalInput")

    din("xT", (BPC, C, NP))
    din("xn", (BPC, NP, C))
    din("bioT", (C, BPC))
    din("confb", (128, BPC), F32)
    din("confT", (BPC, 1), F32)
    din("offsT", (1, 3, BPC * M), F32)
    din("bcoordsT", (1, 3, BPC * M), F32)
    din("w_qkT", (C, 2 * C))
    din("w_vT", (C, VW))
    din("w_caqT", (C, C))
    din("w_cak", (C, C))
    din("w_cavT", (C, C))
    din("w_caoT", (C, C))
    din("w_dbaqT", (C, C))
    din("w_dbakvT", (C, 2 * C))  # sp_w folded in on host
    din("w_dbaoT", (C, C))
    din("w_g1T", (C, C))
    din("w_g2T", (C, C))
    din("w_g3T", (C, C))
    din("w_pT", (C, C))
    din("b_qk", (2 * C,), F32)
    din("b_vw", (1, VW), F32)
    din("b_vwz", (128, VW), F32)
    din("b_stack", (8, C), F32)  # caq cav cao dbaq dbav dbao sp p
    din("b_g", (BPC, C), F32)
    din("ident", (128, 128), F32)
    din("ngrid", (512, 3), F32)
    din("splitT", (2, 128))
    din("hsplit", (128, 2))

    out = nc.dram_tensor("outT", [BPC, C, N], F32, kind="ExternalOutput")

    with tile.TileContext(nc) as tc:
        emit(nc, tc, dram, out)
    nc.compile()
    return dram, out


def emit(nc, tc, dram, out):
    sync, vec, act, gp, pe = nc.sync, nc.vector, nc.scalar, nc.gpsimd, nc.tensor

    def wload(pool, wname, cols=C, colofs=0, name=None, eng=sync):
        t = pool.tile([128, KC, cols], BF16, tag="wbig", name=name or f"w_{wname}")
        src = dram[wname].ap()
        if cols != src.shape[1] or colofs:
            src = src[:, colofs : colofs + cols]
        eng.dma_start(out=t, in_=src.rearrange("(k p) m -> p k m", p=128))
        return t

    with tc.tile_pool(name="consts", bufs=1) as consts, tc.tile_pool(
        name="persist", bufs=1
    ) as persist, tc.tile_pool(name="wpool", bufs=4) as wpool:
        # ---------- big input DMAs first (earliest PE start) ----------
        with tc.tile_pool(name="xpool", bufs=1) as xpool, tc.tile_pool(
            name="smallA", bufs=1
        ) as smA, tc.tile_pool(name="ppA", bufs=2, space="PSUM") as ppA, tc.tile_pool(
            name="ppS", bufs=2, space="PSUM"
        ) as ppS:
            xT = xpool.tile([128, BPC, KC, NP], BF16)
            sync.dma_start(
                out=xT[:, 0],
                in_=dram["xT"].ap()[0].rearrange("(k p) n -> p k n", p=128),
            )
            wqk0 = wload(wpool, "w_qkT", cols=C, colofs=0, name="wqk0")
            wqk1 = wload(wpool, "w_qkT", cols=C, colofs=C, name="wqk1")
            sync.dma_start(
                out=xT[:, 1],
                in_=dram["xT"].ap()[1].rearrange("(k p) n -> p k n", p=128),
            )

            # ---------- small consts on other queues ----------
            # offsets/coords first: the trilinear vec chain waits on them
            offsT0 = consts.tile([1, 3, BPC * M], F32, name="offsT0")
            gp.dma_start(out=offsT0, in_=dram["offsT"].ap())
            baseT0 = consts.tile([1, 3, BPC * M], F32, name="baseT0")
            gp.dma_start(out=baseT0, in_=dram["bcoordsT"].ap())
            ngrid = consts.tile([128, 4, 3], F32)
            gp.dma_start(
                out=ngrid, in_=dram["ngrid"].ap().rearrange("(t p) d -> p t d", p=128)
            )
            ident = consts.tile([128, 128], F32)
            gp.dma_start(out=ident, in_=dram["ident"].ap())
            splitT = consts.tile([2, 128], BF16)
            gp.dma_start(out=splitT, in_=dram["splitT"].ap())
            hsplit = consts.tile([128, 2], BF16)
            gp.dma_start(out=hsplit, in_=dram["hsplit"].ap())
            bioT = consts.tile([128, KC, BPC], BF16)
            gp.dma_start(
                out=bioT, in_=dram["bioT"].ap().rearrange("(k p) b -> p k b", p=128)
            )
            confb = consts.tile([128, BPC], F32)
            gp.dma_start(out=confb, in_=dram["confb"].ap())
            confT = consts.tile([BPC, 1], F32)
            gp.dma_start(out=confT, in_=dram["confT"].ap())

            bqk = consts.tile([128, 12], F32, name="bc_qk")
            gp.dma_start(
                out=bqk, in_=dram["b_qk"].ap().rearrange("(k p) -> p k", p=128)
            )
            bstack = consts.tile([128, 8, KC], F32, name="bc_stack")
            gp.dma_start(
                out=bstack, in_=dram["b_stack"].ap().rearrange("s (k p) -> p s k", p=128)
            )
            bcaq, bcav, bcao = bstack[:, 0], bstack[:, 1], bstack[:, 2]
            bdbaq, bdbav, bdbao = bstack[:, 3], bstack[:, 4], bstack[:, 5]
            bsp, bp = bstack[:, 6], bstack[:, 7]
            bg = consts.tile([BPC, C], F32)
            gp.dma_start(out=bg, in_=dram["b_g"].ap())
            bvw = consts.tile([128, VW], F32)
            gp.dma_start(out=bvw, in_=dram["b_vw"].ap().broadcast_to((128, VW)))
            bvwz = consts.tile([128, VW], F32)
            gp.dma_start(out=bvwz, in_=dram["b_vwz"].ap())

            # persistent activations / small intermediates
            qkT = persist.tile([128, BPC, 12, NP], BF16)
            vn = persist.tile([128, BPC, 5, VW], BF16)
            pixb = persist.tile([128, 3, BPC * M], F32)
            wtri = persist.tile([128, 4, BPC * M], BF16)
            sampT = persist.tile([128, KC, BPC * M], BF16)
            kvdba = persist.tile([128, 12, BPC * M], F32)
            qcaT = persist.tile([128, KC, BPC], BF16)
            qdbaT = persist.tile([128, KC, BPC], F32)
            qmask = persist.tile([128, KC, H * BPC], BF16)
            qkc = persist.tile([128, KC, H, BPC], BF16)
            xbarT = persist.tile([128, KC, BPC * H], BF16)
            ctxcaT = persist.tile([128, KC, BPC], BF16)
            condT = persist.tile([128, KC, BPC], BF16)
            ctxdT = persist.tile([128, KC, BPC], BF16)
            dbaT = persist.tile([128, KC, BPC], BF16)
            bioc = persist.tile([128, KC, BPC], F32)
            rowtc = persist.tile([128, KC, BPC], F32)

            xg = xpool.tile([128, BPC, 4, C], BF16)
            xa = xpool.tile([1, BPC, C], BF16)
            act.dma_start(
                out=xa, in_=dram["xn"].ap()[:, 0:1, :].rearrange("b o c -> o b c")
            )
            for b in range(BPC):
                act.dma_start(
                    out=xg[:, b],
                    in_=dram["xn"]
                    .ap()[b, 1:513, :]
                    .rearrange("(t p) c -> p t c", p=128),
                )

            # --- trilinear hat weights (vector/scalar engines only) ---
            pixT = smA.tile([1, 3, BPC * M], F32)
            vec.tensor_tensor(out=pixT, in0=offsT0, in1=baseT0, op=ALU.add)
            vec.tensor_scalar(pixT, pixT, -1.0, 1.0, ALU.max, ALU.min)
            vec.tensor_scalar(pixT, pixT, 3.5, 3.5, ALU.mult, ALU.add)
            for d in range(3):
                gp.partition_broadcast(pixb[:, d], pixT[:, d, :])
            wd = smA.tile([128, 3, BPC * M], F32)
            wtmp = smA.tile([128, BPC * M], F32)
            for t in range(4):
                for d in range(3):
                    act.activation(
                        wd[:, d], pixb[:, d], AF.Abs, bias=ngrid[:, t, d : d + 1]
                    )
                    act.activation(wd[:, d], wd[:, d], AF.Relu, bias=1.0, scale=-1.0)
                vec.tensor_tensor(out=wtmp, in0=wd[:, 0], in1=wd[:, 1], op=ALU.mult)
                vec.tensor_tensor(
                    out=wtri[:, t], in0=wtmp, in1=wd[:, 2], op=ALU.mult
                )

            # --- P1: q/k projections, channel-major, free 2x260 ---
            for b in range(BPC):
                for m in range(12):
                    w = wqk0 if m < KC else wqk1
                    mo = m % KC
                    ps = ppA.tile([128, 2, 512], F32, tag="pbig")
                    for j in range(2):
                        for k in range(KC):
                            pe.matmul(
                                ps[:, j, :JC],
                                w[:, k, mo * 128 : (mo + 1) * 128],
                                xT[:, b, k, j * JC : (j + 1) * JC],
                                start=(k == 0),
                                stop=(k == KC - 1),
                            )
                    vec.tensor_scalar_add(
                        qkT[:, b, m, :].rearrange("p (j f) -> p j f", j=2),
                        ps[:, :, :JC],
                        bqk[:, m : m + 1],
                    )

            # weights for later phases (deep prefetch, spread queues)
            wva = wpool.tile([128, KC, 512], BF16, tag="wbig", name="wva")
            sync.dma_start(
                out=wva,
                in_=dram["w_vT"].ap()[:, 0:512].rearrange("(k p) m -> p k m", p=128),
            )
            wvb = wpool.tile([128, KC, 268], BF16, tag="wbig", name="wvb")
            sync.dma_start(
                out=wvb,
                in_=dram["w_vT"].ap()[:, 512:780].rearrange("(k p) m -> p k m", p=128),
            )
            wcaq = wload(wpool, "w_caqT")
            wdbaq = wload(wpool, "w_dbaqT")
            wcak = wload(wpool, "w_cak")

            # --- bio-chain stage 1: q_ca (folded) and q_dba ---
            def proj_small(w, rhs_tile, bias, o):
                nf = rhs_tile.shape[-1]
                for mo in range(KC):
                    ps = ppS.tile([128, nf], F32, tag="psm", name="ps_proj")
                    for k in range(KC):
                        pe.matmul(
                            ps,
                            w[:, k, mo * 128 : (mo + 1) * 128],
                            rhs_tile[:, k, :],
                            start=(k == 0),
                            stop=(k == KC - 1),
                        )
                    if bias is None:
                        vec.tensor_copy(out=o[:, mo], in_=ps)
                    else:
                        vec.tensor_scalar_add(o[:, mo], ps, bias[:, mo : mo + 1])

            proj_small(wcaq, bioT, bcaq, qcaT)
            proj_small(wdbaq, bioT, bdbaq, qdbaT)

            # qk-trick via masked q_ca: qmask[p,kk,(h,b)] = qca[p,kk,b] iff
            # h == 2*kk + p//64 else 0; then qkc = sum_kk wcak_kk^T @ qmask_kk
            vec.memset(qmask, 0.0)
            for kk in range(KC):
                for hh in range(2):
                    h = 2 * kk + hh
                    vec.tensor_copy(
                        out=qmask[64 * hh : 64 * hh + 64, kk, h * BPC : (h + 1) * BPC],
                        in_=qcaT[64 * hh : 64 * hh + 64, kk, :],
                    )
            # filler: v projection chunk (b0,t0) while qmask vec ops run
            vdone = set()

            def v_chunk(b, t):
                vdone.add((b, t))
                ps = ppA.tile([128, VW], F32, tag="pbig")
                for (lo, sz), wv in ((NVW[0], wva), (NVW[1], wvb)):
                    for k in range(KC):
                        pe.matmul(
                            ps[:TC, lo : lo + sz],
                            xT[:, b, k, t * TC : t * TC + TC],
                            wv[:, k, :sz],
                            start=(k == 0),
                            stop=(k == KC - 1),
                        )
                # t=4: rows 97..103 are padded tokens (x=0 -> psum 0); bvwz
                # has zero bias there so vn pad rows stay exactly zero.
                bias = bvw if t < 4 else bvwz
                vec.tensor_tensor(
                    out=vn[:TC, b, t, :], in0=ps[:TC], in1=bias[:TC], op=ALU.add
                )

            v_chunk(0, 0)

            for mo in range(KC):
                ps = ppS.tile([128, H * BPC], F32, tag="psm")
                for kk in range(KC):
                    pe.matmul(
                        ps,
                        wcak[:, kk, mo * 128 : (mo + 1) * 128],
                        qmask[:, kk, :],
                        start=(kk == 0),
                        stop=(kk == KC - 1),
                    )
                vec.tensor_copy(out=qkc[:, mo], in_=ps.rearrange("p (h b) -> p h b", b=BPC))

            # --- CA scores (keys 0..512 real; pad cols never exp'd) ---
            caps = []
            for b in range(BPC):
                ps = ppS.tile([H, 2, 512], F32, tag="psm", name=f"caps{b}")
                for j in range(2):
                    for k in range(KC):
                        pe.matmul(
                            ps[:, j, :JC],
                            qkc[:, k, :, b],
                            xT[:, b, k, j * JC : (j + 1) * JC],
                            start=(k == 0),
                            stop=(k == KC - 1),
                        )
                caps.append(ps)
            v_chunk(0, 1)
            v_chunk(0, 2)

            wcav = wload(wpool, "w_cavT")
            wcao = wload(wpool, "w_caoT")

            # --- CA softmax + xbar per batch, v-chunks as PE filler ---
            for b in range(BPC):
                ps = caps[b]
                attn = smA.tile([H, NP], F32, tag="attnca", bufs=2, name="attnca")
                den = smA.tile([H, 2], F32, tag="denca", bufs=2, name="denca")
                act.activation(
                    attn[:, 0:JC], ps[:, 0, :JC], AF.Exp, scale=ISC,
                    accum_out=den[:, 0:1],
                )
                act.activation(
                    attn[:, JC:NREAL], ps[:, 1, 0 : NREAL - JC], AF.Exp, scale=ISC,
                    accum_out=den[:, 1:2],
                )
                vec.tensor_tensor(
                    out=den[:, 0:1], in0=den[:, 0:1], in1=den[:, 1:2], op=ALU.add
                )
                vec.reciprocal(den[:, 0:1], den[:, 0:1])
                vec.tensor_scalar_mul(attn[:, :NREAL], attn[:, :NREAL], den[:, 0:1])
                if b == 0:
                    v_chunk(0, 3)
                    v_chunk(0, 4)
                else:
                    v_chunk(1, 0)
                    v_chunk(1, 1)
                attnT = smA.tile([128, 5, H], BF16, tag="attnT", bufs=2, name="attnT")
                pst0 = ppS.tile([1, H], F32, tag="psm")
                pe.transpose(pst0, attn[:, 0:1], ident[:H, :H])
                vec.tensor_copy(out=attnT[0:1, 0, :], in_=pst0)
                for t in range(4):
                    pst = ppS.tile([128, H], F32, tag="psm")
                    pe.transpose(
                        pst, attn[:, 1 + 128 * t : 1 + 128 * (t + 1)], ident[:H, :H]
                    )
                    vec.tensor_copy(out=attnT[:, 1 + t, :], in_=pst)
                psx = ppS.tile([H, C], F32, tag="psm")
                for lo, sz in N768:
                    pe.matmul(
                        psx[:, lo : lo + sz],
                        attnT[0:1, 0, :],
                        xa[:, b, lo : lo + sz],
                        start=True,
                        stop=False,
                    )
                    for t in range(4):
                        pe.matmul(
                            psx[:, lo : lo + sz],
                            attnT[:, 1 + t, :],
                            xg[:, b, t, lo : lo + sz],
                            start=False,
                            stop=(t == 3),
                        )
                xbar = smA.tile([H, C], F32, tag="xbarca", bufs=2, name="xbarca")
                vec.tensor_copy(out=xbar, in_=psx)
                for k in range(KC):
                    pst = ppS.tile([128, H], F32, tag="psm")
                    pe.transpose(pst, xbar[:, k * 128 : (k + 1) * 128], ident[:H, :H])
                    vec.tensor_copy(out=xbarT[:, k, b * H : (b + 1) * H], in_=pst)

            for b in range(BPC):
                for t in range(5):
                    if (b, t) not in vdone:
                        v_chunk(b, t)

            # --- branch3 sampled = wtri @ patch ---
            samp = smA.tile([M, BPC, C], F32)
            for b in range(BPC):
                ps = ppS.tile([M, C], F32, tag="psm")
                for lo, sz in N768:
                    for t in range(4):
                        pe.matmul(
                            ps[:, lo : lo + sz],
                            wtri[:, t, b * M : (b + 1) * M],
                            xg[:, b, t, lo : lo + sz],
                            start=(t == 0),
                            stop=(t == 3),
                        )
                vec.tensor_copy(out=samp[:, b], in_=ps)

            for b in range(BPC):
                for k in range(KC):
                    pst = ppS.tile([128, M], F32, tag="psm")
                    pe.transpose(
                        pst, samp[:, b, k * 128 : (k + 1) * 128], ident[:M, :M]
                    )
                    vec.tensor_copy(out=sampT[:, k, b * M : (b + 1) * M], in_=pst)

            wdkv0 = wload(wpool, "w_dbakvT", cols=C, colofs=0, name="wdkv0")
            wdkv1 = wload(wpool, "w_dbakvT", cols=C, colofs=C, name="wdkv1")
            wdbao = wload(wpool, "w_dbaoT")
            wg2 = wload(wpool, "w_g2T")
            wg3 = wload(wpool, "w_g3T")

            for m in range(12):
                w = wdkv0 if m < KC else wdkv1
                mo = m % KC
                ps = ppS.tile([128, BPC * M], F32, tag="psm")
                for k in range(KC):
                    pe.matmul(
                        ps,
                        w[:, k, mo * 128 : (mo + 1) * 128],
                        sampT[:, k, :],
                        start=(k == 0),
                        stop=(k == KC - 1),
                    )
                if m < KC:
                    vec.tensor_copy(out=kvdba[:, m], in_=ps)
                else:
                    vec.tensor_scalar_add(
                        kvdba[:, m], ps, bdbav[:, (m - KC) : (m - KC) + 1]
                    )

            # --- dba tiny attention (vec-heavy; ctx/cond matmuls fill PE) ---
            prod = smA.tile([128, BPC * M], BF16, name="prodb")
            prodf = smA.tile([128, BPC * M], F32, name="prodf")
            edba = smA.tile([2, KC, BPC * M], BF16)
            ddba = smA.tile([2, KC, BPC], F32)
            for kk in range(KC):
                vec.tensor_tensor(
                    out=prod.rearrange("p (b m) -> p b m", b=BPC),
                    in0=kvdba[:, kk, :].rearrange("p (b m) -> p b m", b=BPC),
                    in1=qdbaT[:, kk, :].unsqueeze(2).broadcast_to((128, BPC, M)),
                    op=ALU.mult,
                )
                pd = ppS.tile([2, BPC * M], F32, tag="psm", name="pd_sdba")
                pe.matmul(pd, hsplit, prod, start=True, stop=True)
                act.activation(edba[:, kk, :], pd, AF.Exp, scale=ISC)

            # ctx_ca: full 24-col matmul then block-select (+bias)
            for mo in range(KC):
                ps = ppS.tile([128, BPC * H], F32, tag="psm")
                for k in range(KC):
                    pe.matmul(
                        ps,
                        wcav[:, k, mo * 128 : (mo + 1) * 128],
                        xbarT[:, k, :],
                        start=(k == 0),
                        stop=(k == KC - 1),
                    )
                for hh in range(2):
                    h = 2 * mo + hh
                    vec.tensor_scalar_add(
                        ctxcaT[64 * hh : 64 * hh + 64, mo, :],
                        ps[64 * hh : 64 * hh + 64, h : BPC * H : H],
                        bcav[64 * hh : 64 * hh + 64, mo : mo + 1],
                    )

            vec.tensor_reduce(
                ddba,
                edba.rearrange("p k (b m) -> p k b m", b=BPC),
                axis=AX.X,
                op=ALU.add,
            )
            vec.reciprocal(ddba, ddba)
            for kk in range(KC):
                for b in range(BPC):
                    vec.tensor_scalar_mul(
                        edba[:, kk, b * M : (b + 1) * M],
                        edba[:, kk, b * M : (b + 1) * M],
                        ddba[:, kk, b : b + 1],
                    )

            proj_small(wcao, ctxcaT, bcao, condT)

            ctxdf = smA.tile([128, KC, BPC], F32, name="ctxdf")
            for kk in range(KC):
                psb = ppS.tile([128, BPC * M], F32, tag="psm")
                pe.matmul(psb, splitT, edba[:, kk, :], start=True, stop=True)
                vec.tensor_tensor(
                    out=prodf, in0=kvdba[:, KC + kk, :], in1=psb, op=ALU.mult
                )
                vec.tensor_reduce(
                    ctxdf[:, kk, :],
                    prodf.rearrange("p (b m) -> p b m", b=BPC),
                    axis=AX.X,
                    op=ALU.add,
                )
            vec.tensor_copy(out=ctxdT, in_=ctxdf)

            # psg2 rides here (PE filler under dba vec work)
            rowg2 = smA.tile([BPC, C], F32, name="rowg2")
            psg2 = ppS.tile([BPC, C], F32, tag="psm", name="psg2")
            for lo, sz in N768:
                for k in range(KC):
                    pe.matmul(
                        psg2[:, lo : lo + sz],
                        condT[:, k, :],
                        wg2[:, k, lo : lo + sz],
                        start=(k == 0),
                        stop=(k == KC - 1),
                    )
            vec.tensor_copy(out=rowg2, in_=psg2)

            proj_small(wdbao, ctxdT, bdbao, dbaT)

            # bio path column: conf*(0.5*cond + 0.5*conf*dba)
            for b in range(BPC):
                vec.tensor_scalar_mul(
                    bioc[:, :, b : b + 1], dbaT[:, :, b : b + 1], confb[:, b : b + 1]
                )
            vec.tensor_tensor(out=bioc, in0=bioc, in1=condT, op=ALU.add)
            for b in range(BPC):
                vec.tensor_scalar(
                    bioc[:, :, b : b + 1],
                    bioc[:, :, b : b + 1],
                    confb[:, b : b + 1],
                    0.5,
                    ALU.mult,
                    ALU.mult,
                )

            # gate row terms: rowt = psg3*conf + psg2 + b_g
            psg3 = ppS.tile([BPC, C], F32, tag="psm", name="psg3")
            for lo, sz in N768:
                for k in range(KC):
                    pe.matmul(
                        psg3[:, lo : lo + sz],
                        dbaT[:, k, :],
                        wg3[:, k, lo : lo + sz],
                        start=(k == 0),
                        stop=(k == KC - 1),
                    )
            rowf = smA.tile([BPC, C], F32, name="rowf")
            vec.tensor_scalar_mul(rowf, psg3, confT)
            vec.tensor_tensor(out=rowf, in0=rowf, in1=rowg2, op=ALU.add)
            vec.tensor_tensor(out=rowf, in0=rowf, in1=bg, op=ALU.add)
            # transpose to column layout [c, b] for use as sigmoid bias
            for k in range(KC):
                pst = ppS.tile([128, BPC], F32, tag="psm")
                pe.transpose(pst, rowf[:, k * 128 : (k + 1) * 128], ident[:BPC, :BPC])
                vec.tensor_copy(out=rowtc[:, k, :], in_=pst)

        # ---------------- window 2: self-attention ----------------
        with tc.tile_pool(name="soPool", bufs=1) as soP:
            soT = soP.tile([128, BPC, KC, NP], BF16)
            with tc.tile_pool(name="smallB", bufs=1) as smB, tc.tile_pool(
                name="ppB", bufs=2, space="PSUM"
            ) as ppB:
                pairs = [(b, h) for b in range(BPC) for h in range(H)]

                def scores_chunk(b, h, t, etp):
                    """scores t-chunk + its exp into slot t%2 of etp[t//2]."""
                    po, mq, mk = 64 * (h % 2), h // 2, 6 + h // 2
                    pss = ppB.tile([128, 2, 512], F32, tag="psc", bufs=3)
                    for j in range(2):
                        pe.matmul(
                            pss[:TC, j, :JC],
                            qkT[po : po + 64, b, mk, t * TC : t * TC + TC],
                            qkT[po : po + 64, b, mq, j * JC : (j + 1) * JC],
                            start=True,
                            stop=True,
                        )
                    act.activation(
                        etp[t // 2][:TC, t % 2],
                        pss[:TC, :, :JC],
                        AF.Exp,
                        scale=ISC,
                    )

                def av_chunk(b, h, t, etp, psc):
                    for j in range(2):
                        pe.matmul(
                            psc[:, j, :JC],
                            vn[:TC, b, t, 65 * h : 65 * h + 65],
                            etp[t // 2][:TC, t % 2, j, :],
                            start=(t == 0),
                            stop=(t == 4),
                        )

                def finalize(b, h, psc):
                    po, mq = 64 * (h % 2), h // 2
                    den = smB.tile([1, NP], F32, tag="den", bufs=2, name="den")
                    vec.tensor_copy(
                        out=den.rearrange("p (j f) -> p j f", j=2),
                        in_=psc[64:65, :, :JC],
                    )
                    rec = smB.tile([1, NP], F32, tag="rec", bufs=2, name="rec")
                    vec.reciprocal_approx_fast(rec, den)
                    rbc = smB.tile([64, NP], F32, tag="rbc", bufs=2, name="rbc")
                    gp.partition_broadcast(rbc, rec)
                    vec.tensor_tensor(
                        out=soT[po : po + 64, b, mq, :].rearrange(
                            "p (j f) -> p j f", j=2
                        ),
                        in0=psc[:64, :, :JC],
                        in1=rbc.rearrange("p (j f) -> p j f", j=2),
                        op=ALU.mult,
                    )

                # software pipeline: scores(i) chunks interleaved with
                # attn@v(i-1) chunks so no PE instruction ever waits on the
                # scalar-engine exp stream (stalls reset the PE clock ramp).
                prev = None  # (b, h, etp, psc)
                for bh in pairs + [None]:
                    cur = None
                    if bh is not None:
                        b, h = bh
                        psc = ppB.tile([65, 2, 512], F32, tag="pcx", bufs=1)
                        etp = [
                            smB.tile(
                                [128, 2, 2, JC], BF16, tag="expT", bufs=8, name="expT"
                            )
                            for _ in range(3)
                        ]
                        for t in range(5):
                            scores_chunk(b, h, t, etp)
                    if bh is not None:
                        cur = (b, h, etp, psc)
                    if prev is not None:
                        for t in range(5):
                            av_chunk(prev[0], prev[1], t, prev[2], prev[3])
                        finalize(prev[0], prev[1], prev[3])
                    prev = cur

            # ---------------- window 3: gate + fusion + proj ----------------
            with tc.tile_pool(name="smallC", bufs=1) as smC, tc.tile_pool(
                name="ppC", bufs=2, space="PSUM"
            ) as ppC:
                wg1 = wload(wpool, "w_g1T", eng=sync)
                wp = wload(wpool, "w_pT", eng=sync)
                fusedT = [None, None]
                for b in range(BPC):
                    fusedT[b] = smC.tile(
                        [128, KC, NP], BF16, tag=f"fusedT{b}", bufs=1, name="fusedT"
                    )
                    for mo in range(KC):
                        ps = ppC.tile([128, 2, 512], F32, tag="pgate")
                        for j in range(2):
                            for k in range(KC):
                                pe.matmul(
                                    ps[:, j, :JC],
                                    wg1[:, k, mo * 128 : (mo + 1) * 128],
                                    soT[:, b, k, j * JC : (j + 1) * JC],
                                    start=(k == 0),
                                    stop=(k == KC - 1),
                                )
                        gateT = smC.tile(
                            [128, NP], BF16, tag="gateT", bufs=2, name="gateT"
                        )
                        act.activation(
                            gateT.rearrange("p (j f) -> p j f", j=2),
                            ps[:, :, :JC],
                            AF.Sigmoid,
                            bias=rowtc[:, mo, b : b + 1],
                        )
                        vec.scalar_tensor_tensor(
                            out=fusedT[b][:, mo, :],
                            in0=soT[:, b, mo, :],
                            scalar=bioc[:, mo, b : b + 1],
                            in1=gateT,
                            op0=ALU.subtract,
                            op1=ALU.mult,
                        )
                        vec.tensor_tensor(
                            out=fusedT[b][:, mo, :],
                            in0=soT[:, b, mo, :],
                            in1=fusedT[b][:, mo, :],
                            op=ALU.subtract,
                        )
                for b in range(BPC):
                    for mo in range(KC):
                        ps = ppC.tile([128, 2, 512], F32, tag="pgate")
                        for j in range(2):
                            for k in range(KC):
                                pe.matmul(
                                    ps[:, j, :JC],
                                    wp[:, k, mo * 128 : (mo + 1) * 128],
                                    fusedT[b][:, k, j * JC : (j + 1) * JC],
                                    start=(k == 0),
                                    stop=(k == KC - 1),
                                )
                        outT = smC.tile([128, NP], F32, tag="outT", bufs=3, name="oT")
                        vec.tensor_scalar_add(
                            outT.rearrange("p (j f) -> p j f", j=2),
                            ps[:, :, :JC],
                            bp[:, mo : mo + 1],
                        )
                        eng = sync if mo % 2 == 0 else act
                        eng.dma_start(
                            out=out.ap()[b, mo * 128 : (mo + 1) * 128, :],
                            in_=outT[:, 0:NREAL],
                        )


# ====================== host side ======================


def stage_inputs(inputs):
    """Pure layout/dtype staging of the full inputs into 8 per-core in_maps."""
    f = np.float32
    bf = ml_dtypes.bfloat16
    x = np.asarray(inputs["x"], f)
    bio = np.asarray(inputs["bio_embed"], f)
    conf = np.asarray(inputs["confidence"], f)
    bco = np.asarray(inputs["base_coords"], f)
    offs = np.asarray(inputs["offsets"], f)

    W = {k: np.asarray(v, f) for k, v in inputs.items()}
    qkv_w = W["qkv_w"]
    qkv_b = W["qkv_b"]
    wv = qkv_w[2 * C :]
    w_vT = np.zeros((C, VW), f)
    b_vw = np.zeros((1, VW), f)
    for h in range(H):
        w_vT[:, 65 * h : 65 * h + 64] = wv[64 * h : 64 * h + 64].T
        b_vw[0, 65 * h : 65 * h + 64] = qkv_b[2 * C + 64 * h : 2 * C + 64 * h + 64]
        b_vw[0, 65 * h + 64] = 1.0

    ident = np.eye(128, dtype=f)
    gz, gy, gx = np.meshgrid(np.arange(G), np.arange(G), np.arange(G), indexing="ij")
    ngrid = -np.stack([gx.ravel(), gy.ravel(), gz.ravel()], axis=1).astype(f)
    splitT = np.zeros((2, 128), f)
    splitT[0, :64] = 1.0
    splitT[1, 64:] = 1.0

    # fold bio_query into ca_query: q_ca = Wcaq @ (Wbioq @ bio + b_bioq) + b_caq
    w_caq = W["ca_in_w"][:C]
    w_caq_eff = w_caq @ W["bio_query_w"]
    b_caq_eff = w_caq @ W["bio_query_b"] + W["ca_in_b"][:C]

    shared = {
        "w_qkT": qkv_w[: 2 * C].T,
        "w_vT": w_vT,
        "w_caqT": w_caq_eff.T,
        "w_cak": W["ca_in_w"][C : 2 * C],
        "w_cavT": W["ca_in_w"][2 * C :].T,
        "w_caoT": W["ca_out_w"].T,
        "w_dbaqT": W["dba_in_w"][:C].T,
        "w_dbakvT": (W["dba_in_w"][C:] @ W["sp_w"]).T,
        "w_dbaoT": W["dba_out_w"].T,
        "w_g1T": W["gate_w"][:, :C].T,
        "w_g2T": W["gate_w"][:, C : 2 * C].T,
        "w_g3T": W["gate_w"][:, 2 * C :].T,
        "w_pT": W["proj_w"].T,
        "splitT": splitT,
        "hsplit": splitT.T,
    }
    shared = {k: np.ascontiguousarray(v, bf) for k, v in shared.items()}
    b_vwz = np.broadcast_to(b_vw, (128, VW)).copy()
    b_vwz[97:] = 0.0

    b_stack = np.stack(
        [
            b_caq_eff,
            W["ca_in_b"][2 * C :],
            W["ca_out_b"],
            W["dba_in_b"][:C],
            W["dba_in_w"][2 * C :] @ W["sp_b"] + W["dba_in_b"][2 * C :],
            W["dba_out_b"],
            np.zeros(C, np.float32),
            W["proj_b"],
        ]
    )
    shared_f32 = {
        "b_qk": qkv_b[: 2 * C],
        "b_vw": b_vw,
        "b_vwz": b_vwz,
        "b_stack": b_stack,
        "b_g": np.broadcast_to(W["gate_b"].reshape(1, C), (BPC, C)),
        "ident": ident,
        "ngrid": ngrid,
    }
    shared.update(
        {k: np.ascontiguousarray(v, f) for k, v in shared_f32.items()}
    )
    shared["bcoordsT"] = np.ascontiguousarray(
        np.broadcast_to(bco.T[:, None, :], (3, BPC, M)).reshape(1, 3, BPC * M), f
    )

    xpad = np.zeros((B, NP, C), f)
    xpad[:, :NREAL] = x

    in_maps = []
    for c in range(NCORES):
        sl = slice(c * BPC, (c + 1) * BPC)
        m = dict(shared)
        m["xn"] = np.ascontiguousarray(xpad[sl], bf)
        m["xT"] = np.ascontiguousarray(xpad[sl].transpose(0, 2, 1), bf)
        m["bioT"] = np.ascontiguousarray(bio[sl].T, bf)
        m["confb"] = np.ascontiguousarray(
            np.broadcast_to(conf[sl].reshape(1, BPC), (128, BPC)), f
        )
        m["confT"] = np.ascontiguousarray(conf[sl].reshape(BPC, 1), f)
        m["offsT"] = np.ascontiguousarray(
            offs[sl].transpose(2, 0, 1).reshape(1, 3, BPC * M), f
        )
        in_maps.append(m)
    return in_maps


_CACHE = {}


def get_nc():
    if "nc" not in _CACHE:
        nc = bacc.Bacc("TRN2", target_bir_lowering=False, debug=False)
        build(nc)
        _CACHE["nc"] = nc
    return _CACHE["nc"]


def _ensure_ntff_hook():
    """The agent image's antenv lacks axon_hooks; shim it so trace=True can
    reach the libaxon NTFF profiler (profiling only, test-harness use)."""
    import types

    try:
        import antenv.axon_hooks  # noqa: F401

        return
    except ImportError:
        pass
    mod = types.ModuleType("antenv.axon_hooks")
    state = {"h": None}
    mod.set_axon_ntff_profile_hook = lambda h: state.__setitem__("h", h)
    mod.get_axon_ntff_profile_hook = lambda: state["h"]
    sys.modules["antenv.axon_hooks"] = mod
    import antenv

    antenv.axon_hooks = mod
    try:
        from trn_agent_boot.trn_boot import _ntff_profile_via_ctypes

        hook = _ntff_profile_via_ctypes("/opt/axon/libaxon_pjrt.so")
        if hook is not None:
            mod.set_axon_ntff_profile_hook(hook)
    except Exception:
        pass


def kernel(**inputs):
    trace = bool(int(os.environ.get("KERNEL_TRACE", "0")))
    if trace:
        _ensure_ntff_hook()
    nc = get_nc()
    in_maps = stage_inputs(inputs)
    res = bass_utils.run_bass_kernel_spmd(
        nc,
        in_maps,
        core_ids=list(range(NCORES)),
        trace=trace,
    )
    _CACHE["last_result"] = res
    outT = np.stack([res.results[c]["outT"] for c in range(NCORES)])
    out = outT.reshape(B, C, N).transpose(0, 2, 1)
    return np.ascontiguousarray(out, dtype=np.float32)


# revision 63
# speedup vs baseline: 1.2573x; 1.2573x over previous
"""Trainium2 Bass kernel for BiomarkerConditionedAttention.

Sharding: pure data-parallel over batch B=16 across 8 cores (2 batches/core).

v2 layout strategy (per core; "T" = feature-on-partitions):
  - all matmul operands bf16 (PSUM accum f32); error budget 2e-2 allows it.
  - tokens padded 513 -> 520 on host (zero pad): every 513-free matmul
    becomes 2x260 PSUM-bank chunks - no 1-wide companion matmuls.
    Padded KEYS are neutralized by zeroing their vn rows (v and the
    ones-column), so they contribute 0 to both context and denominator.
    Padded QUERY columns produce garbage that is sliced off at the end.
  - branch1: q/k channel-major; v token-major widened with a per-head
    ones-column so attn@v also emits the softmax denominator row.
  - branch2 cross-attn via rank-1 algebra; bio_query projection folded
    into ca_query on host (W_caq_eff = W_caq @ W_bioq).
  - qk-trick and ca-value use masked/full-then-select batching to cut
    tiny-matmul count.
  - branch3 grid_sample as dense matmul with on-device trilinear weights.
  - gate: per-token part contracts only standard_out; conditioned/dba
    slabs are per-batch row terms added via a rank-1 selector matmul.
"""

import os
import sys

sys.path.insert(0, "/opt/trn_rl_repo")

import ml_dtypes
import numpy as np

import concourse.bass as bass
import concourse.mybir as mybir
import concourse.tile as tile
from concourse import bacc, bass_utils

F32 = mybir.dt.float32
BF16 = mybir.dt.bfloat16
FP8 = mybir.dt.float8e4
DR = mybir.MatmulPerfMode.DoubleRow
AF = mybir.ActivationFunctionType
ALU = mybir.AluOpType
AX = mybir.AxisListType

B, N, C, H, M, G = 16, 513, 768, 12, 5, 8
HD = C // H  # 64
ISC = float(1.0 / np.sqrt(HD))
NCORES = 8
BPC = B // NCORES  # 2
KC = C // 128  # 6
NP = 520  # padded token count
JC = 260  # free-dim chunk (2 per 520)
TC = 104  # token chunk (partition dim), 5 per 520
NREAL = 513
VW = H * (HD + 1)  # 780
N768 = [(0, 512), (512, 256)]
NVW = [(0, 512), (512, 268)]


def build(nc: bass.Bass):
    dram = {}

    def din(name, shape, dt=BF16):
        dram[name] = nc.dram_tensor(name, list(shape), dt, kind="ExternalInput")

    din("xT", (BPC, C, NP))
    din("xn", (BPC, NP, C))
    din("bioT", (C, BPC))
    din("confb", (128, BPC), F32)
    din("confT", (BPC, 1), F32)
    din("offsT", (1, 3, BPC * M), F32)
    din("bcoordsT", (1, 3, BPC * M), F32)
    din("w_qkT", (C, 2 * C))
    din("w_vT", (C, VW))
    din("w_caqT", (C, C))
    din("w_cak", (C, C))
    din("w_cavT", (C, C))
    din("w_caoT", (C, C))
    din("w_dbaqT", (C, C))
    din("w_dbakvT", (C, 2 * C))  # sp_w folded in on host
    din("w_dbaoT", (C, C))
    din("w_g1T", (C, C))
    din("w_g2T", (C, C))
    din("w_g3T", (C, C))
    din("w_pT", (C, C))
    din("b_qk", (2 * C,), F32)
    din("b_vw", (1, VW), F32)
    din("b_vwz", (128, VW), F32)
    din("b_stack", (8, C), F32)  # caq cav cao dbaq dbav dbao sp p
    din("b_g", (BPC, C), F32)
    din("ident", (128, 128), F32)
    din("ngrid", (512, 3), F32)
    din("splitT", (2, 128))
    din("hsplit", (128, 2))

    out = nc.dram_tensor("outT", [BPC, C, N], F32, kind="ExternalOutput")

    with tile.TileContext(nc) as tc:
        emit(nc, tc, dram, out)
    nc.compile()
    return dram, out


def emit(nc, tc, dram, out):
    sync, vec, act, gp, pe = nc.sync, nc.vector, nc.scalar, nc.gpsimd, nc.tensor

    def wload(pool, wname, cols=C, colofs=0, name=None, eng=sync):
        t = pool.tile([128, KC, cols], BF16, tag="wbig", name=name or f"w_{wname}")
        src = dram[wname].ap()
        if cols != src.shape[1] or colofs:
            src = src[:, colofs : colofs + cols]
        eng.dma_start(out=t, in_=src.rearrange("(k p) m -> p k m", p=128))
        return t

    with tc.tile_pool(name="consts", bufs=1) as consts, tc.tile_pool(
        name="persist", bufs=1
    ) as persist, tc.tile_pool(name="wpool", bufs=4) as wpool:
        # ---------- big input DMAs first (earliest PE start) ----------
        with tc.tile_pool(name="xpool", bufs=1) as xpool, tc.tile_pool(
            name="smallA", bufs=1
        ) as smA, tc.tile_pool(name="ppA", bufs=2, space="PSUM") as ppA, tc.tile_pool(
            name="ppS", bufs=2, space="PSUM"
        ) as ppS:
            xT = xpool.tile([128, BPC, KC, NP], BF16)
            sync.dma_start(
                out=xT[:, 0],
                in_=dram["xT"].ap()[0].rearrange("(k p) n -> p k n", p=128),
            )
            wqk0 = wload(wpool, "w_qkT", cols=C, colofs=0, name="wqk0")
            wqk1 = wload(wpool, "w_qkT", cols=C, colofs=C, name="wqk1")
            sync.dma_start(
                out=xT[:, 1],
                in_=dram["xT"].ap()[1].rearrange("(k p) n -> p k n", p=128),
            )

            # ---------- small consts on other queues ----------
            # offsets/coords first: the trilinear vec chain waits on them
            offsT0 = consts.tile([1, 3, BPC * M], F32, name="offsT0")
            gp.dma_start(out=offsT0, in_=dram["offsT"].ap())
            baseT0 = consts.tile([1, 3, BPC * M], F32, name="baseT0")
            gp.dma_start(out=baseT0, in_=dram["bcoordsT"].ap())
            ngrid = consts.tile([128, 4, 3], F32)
            gp.dma_start(
                out=ngrid, in_=dram["ngrid"].ap().rearrange("(t p) d -> p t d", p=128)
            )
            ident = consts.tile([128, 128], F32)
            gp.dma_start(out=ident, in_=dram["ident"].ap())
            splitT = consts.tile([2, 128], BF16)
            gp.dma_start(out=splitT, in_=dram["splitT"].ap())
            hsplit = consts.tile([128, 2], BF16)
            gp.dma_start(out=hsplit, in_=dram["hsplit"].ap())
            bioT = consts.tile([128, KC, BPC], BF16)
            gp.dma_start(
                out=bioT, in_=dram["bioT"].ap().rearrange("(k p) b -> p k b", p=128)
            )
            confb = consts.tile([128, BPC], F32)
            gp.dma_start(out=confb, in_=dram["confb"].ap())
            confT = consts.tile([BPC, 1], F32)
            gp.dma_start(out=confT, in_=dram["confT"].ap())

            bqk = consts.tile([128, 12], F32, name="bc_qk")
            gp.dma_start(
                out=bqk, in_=dram["b_qk"].ap().rearrange("(k p) -> p k", p=128)
            )
            bstack = consts.tile([128, 8, KC], F32, name="bc_stack")
            gp.dma_start(
                out=bstack, in_=dram["b_stack"].ap().rearrange("s (k p) -> p s k", p=128)
            )
            bcaq, bcav, bcao = bstack[:, 0], bstack[:, 1], bstack[:, 2]
            bdbaq, bdbav, bdbao = bstack[:, 3], bstack[:, 4], bstack[:, 5]
            bsp, bp = bstack[:, 6], bstack[:, 7]
            bg = consts.tile([BPC, C], F32)
            gp.dma_start(out=bg, in_=dram["b_g"].ap())
            bvw = consts.tile([128, VW], F32)
            gp.dma_start(out=bvw, in_=dram["b_vw"].ap().broadcast_to((128, VW)))
            bvwz = consts.tile([128, VW], F32)
            gp.dma_start(out=bvwz, in_=dram["b_vwz"].ap())

            # persistent activations / small intermediates
            qkT = persist.tile([128, BPC, 12, NP], BF16)
            vn = persist.tile([128, BPC, 5, VW], BF16)
            pixb = persist.tile([128, 3, BPC * M], F32)
            wtri = persist.tile([128, 4, BPC * M], BF16)
            sampT = persist.tile([128, KC, BPC * M], BF16)
            kvdba = persist.tile([128, 12, BPC * M], F32)
            qcaT = persist.tile([128, KC, BPC], BF16)
            qdbaT = persist.tile([128, KC, BPC], F32)
            qmask = persist.tile([128, KC, H * BPC], BF16)
            qkc = persist.tile([128, KC, H, BPC], BF16)
            xbarT = persist.tile([128, KC, BPC * H], BF16)
            ctxcaT = persist.tile([128, KC, BPC], BF16)
            condT = persist.tile([128, KC, BPC], BF16)
            ctxdT = persist.tile([128, KC, BPC], BF16)
            dbaT = persist.tile([128, KC, BPC], BF16)
            bioc = persist.tile([128, KC, BPC], F32)
            rowtc = persist.tile([128, KC, BPC], F32)

            xg = xpool.tile([128, BPC, 4, C], BF16)
            xa = xpool.tile([1, BPC, C], BF16)
            act.dma_start(
                out=xa, in_=dram["xn"].ap()[:, 0:1, :].rearrange("b o c -> o b c")
            )
            for b in range(BPC):
                act.dma_start(
                    out=xg[:, b],
                    in_=dram["xn"]
                    .ap()[b, 1:513, :]
                    .rearrange("(t p) c -> p t c", p=128),
                )

            # --- trilinear hat weights (vector/scalar engines only) ---
            pixT = smA.tile([1, 3, BPC * M], F32)
            vec.tensor_tensor(out=pixT, in0=offsT0, in1=baseT0, op=ALU.add)
            vec.tensor_scalar(pixT, pixT, -1.0, 1.0, ALU.max, ALU.min)
            vec.tensor_scalar(pixT, pixT, 3.5, 3.5, ALU.mult, ALU.add)
            for d in range(3):
                gp.partition_broadcast(pixb[:, d], pixT[:, d, :])
            wd = smA.tile([128, 3, BPC * M], F32)
            wtmp = smA.tile([128, BPC * M], F32)
            for t in range(4):
                for d in range(3):
                    act.activation(
                        wd[:, d], pixb[:, d], AF.Abs, bias=ngrid[:, t, d : d + 1]
                    )
                    act.activation(wd[:, d], wd[:, d], AF.Relu, bias=1.0, scale=-1.0)
                vec.tensor_tensor(out=wtmp, in0=wd[:, 0], in1=wd[:, 1], op=ALU.mult)
                vec.tensor_tensor(
                    out=wtri[:, t], in0=wtmp, in1=wd[:, 2], op=ALU.mult
                )

            # --- P1: q/k projections, channel-major, free 2x260 ---
            for b in range(BPC):
                for m in range(12):
                    w = wqk0 if m < KC else wqk1
                    mo = m % KC
                    ps = ppA.tile([128, 2, 512], F32, tag="pbig")
                    for j in range(2):
                        for k in range(KC):
                            pe.matmul(
                                ps[:, j, :JC],
                                w[:, k, mo * 128 : (mo + 1) * 128],
                                xT[:, b, k, j * JC : (j + 1) * JC],
                                start=(k == 0),
                                stop=(k == KC - 1),
                            )
                    vec.tensor_scalar_add(
                        qkT[:, b, m, :].rearrange("p (j f) -> p j f", j=2),
                        ps[:, :, :JC],
                        bqk[:, m : m + 1],
                    )

            # weights for later phases (deep prefetch, spread queues)
            wva = wpool.tile([128, KC, 512], BF16, tag="wbig", name="wva")
            sync.dma_start(
                out=wva,
                in_=dram["w_vT"].ap()[:, 0:512].rearrange("(k p) m -> p k m", p=128),
            )
            wvb = wpool.tile([128, KC, 268], BF16, tag="wbig", name="wvb")
            sync.dma_start(
                out=wvb,
                in_=dram["w_vT"].ap()[:, 512:780].rearrange("(k p) m -> p k m", p=128),
            )
            wcaq = wload(wpool, "w_caqT")
            wdbaq = wload(wpool, "w_dbaqT")
            wcak = wload(wpool, "w_cak")

            # --- bio-chain stage 1: q_ca (folded) and q_dba ---
            def proj_small(w, rhs_tile, bias, o):
                nf = rhs_tile.shape[-1]
                for mo in range(KC):
                    ps = ppS.tile([128, nf], F32, tag="psm", name="ps_proj")
                    for k in range(KC):
                        pe.matmul(
                            ps,
                            w[:, k, mo * 128 : (mo + 1) * 128],
                            rhs_tile[:, k, :],
                            start=(k == 0),
                            stop=(k == KC - 1),
                        )
                    if bias is None:
                        vec.tensor_copy(out=o[:, mo], in_=ps)
                    else:
                        vec.tensor_scalar_add(o[:, mo], ps, bias[:, mo : mo + 1])

            proj_small(wcaq, bioT, bcaq, qcaT)
            proj_small(wdbaq, bioT, bdbaq, qdbaT)

            # qk-trick via masked q_ca: qmask[p,kk,(h,b)] = qca[p,kk,b] iff
            # h == 2*kk + p//64 else 0; then qkc = sum_kk wcak_kk^T @ qmask_kk
            vec.memset(qmask, 0.0)
            for kk in range(KC):
                for hh in range(2):
                    h = 2 * kk + hh
                    vec.tensor_copy(
                        out=qmask[64 * hh : 64 * hh + 64, kk, h * BPC : (h + 1) * BPC],
                        in_=qcaT[64 * hh : 64 * hh + 64, kk, :],
                    )
            # filler: v projection chunk (b0,t0) while qmask vec ops run
            vdone = set()

            def v_chunk(b, t):
                vdone.add((b, t))
                ps = ppA.tile([128, VW], F32, tag="pbig")
                for (lo, sz), wv in ((NVW[0], wva), (NVW[1], wvb)):
                    for k in range(KC):
                        pe.matmul(
                            ps[:TC, lo : lo + sz],
                            xT[:, b, k, t * TC : t * TC + TC],
                            wv[:, k, :sz],
                            start=(k == 0),
                            stop=(k == KC - 1),
                        )
                # t=4: rows 97..103 are padded tokens (x=0 -> psum 0); bvwz
                # has zero bias there so vn pad rows stay exactly zero.
                bias = bvw if t < 4 else bvwz
                vec.tensor_tensor(
                    out=vn[:TC, b, t, :], in0=ps[:TC], in1=bias[:TC], op=ALU.add
                )

            v_chunk(0, 0)

            for mo in range(KC):
                ps = ppS.tile([128, H * BPC], F32, tag="psm")
                for kk in range(KC):
                    pe.matmul(
                        ps,
                        wcak[:, kk, mo * 128 : (mo + 1) * 128],
                        qmask[:, kk, :],
                        start=(kk == 0),
                        stop=(kk == KC - 1),
                    )
                vec.tensor_copy(out=qkc[:, mo], in_=ps.rearrange("p (h b) -> p h b", b=BPC))

            # --- CA scores (keys 0..512 real; pad cols never exp'd) ---
            caps = []
            for b in range(BPC):
                ps = ppS.tile([H, 2, 512], F32, tag="psm", name=f"caps{b}")
                for j in range(2):
                    for k in range(KC):
                        pe.matmul(
                            ps[:, j, :JC],
                            qkc[:, k, :, b],
                            xT[:, b, k, j * JC : (j + 1) * JC],
                            start=(k == 0),
                            stop=(k == KC - 1),
                        )
                caps.append(ps)
            v_chunk(0, 1)
            v_chunk(0, 2)

            wcav = wload(wpool, "w_cavT")
            wcao = wload(wpool, "w_caoT")

            # --- CA softmax + xbar per batch, v-chunks as PE filler ---
            for b in range(BPC):
                ps = caps[b]
                attn = smA.tile([H, NP], F32, tag="attnca", bufs=2, name="attnca")
                den = smA.tile([H, 2], F32, tag="denca", bufs=2, name="denca")
                act.activation(
                    attn[:, 0:JC], ps[:, 0, :JC], AF.Exp, scale=ISC,
                    accum_out=den[:, 0:1],
                )
                act.activation(
                    attn[:, JC:NREAL], ps[:, 1, 0 : NREAL - JC], AF.Exp, scale=ISC,
                    accum_out=den[:, 1:2],
                )
                vec.tensor_tensor(
                    out=den[:, 0:1], in0=den[:, 0:1], in1=den[:, 1:2], op=ALU.add
                )
                vec.reciprocal(den[:, 0:1], den[:, 0:1])
                vec.tensor_scalar_mul(attn[:, :NREAL], attn[:, :NREAL], den[:, 0:1])
                if b == 0:
                    v_chunk(0, 3)
                    v_chunk(0, 4)
                else:
                    v_chunk(1, 0)
                    v_chunk(1, 1)
                attnT = smA.tile([128, 5, H], BF16, tag="attnT", bufs=2, name="attnT")
                pst0 = ppS.tile([1, H], F32, tag="psm")
                pe.transpose(pst0, attn[:, 0:1], ident[:H, :H])
                vec.tensor_copy(out=attnT[0:1, 0, :], in_=pst0)
                for t in range(4):
                    pst = ppS.tile([128, H], F32, tag="psm")
                    pe.transpose(
                        pst, attn[:, 1 + 128 * t : 1 + 128 * (t + 1)], ident[:H, :H]
                    )
                    vec.tensor_copy(out=attnT[:, 1 + t, :], in_=pst)
                psx = ppS.tile([H, C], F32, tag="psm")
                for lo, sz in N768:
                    pe.matmul(
                        psx[:, lo : lo + sz],
                        attnT[0:1, 0, :],
                        xa[:, b, lo : lo + sz],
                        start=True,
                        stop=False,
                    )
                    for t in range(4):
                        pe.matmul(
                            psx[:, lo : lo + sz],
                            attnT[:, 1 + t, :],
                            xg[:, b, t, lo : lo + sz],
                            start=False,
                            stop=(t == 3),
                        )
                xbar = smA.tile([H, C], F32, tag="xbarca", bufs=2, name="xbarca")
                vec.tensor_copy(out=xbar, in_=psx)
                for k in range(KC):
                    pst = ppS.tile([128, H], F32, tag="psm")
                    pe.transpose(pst, xbar[:, k * 128 : (k + 1) * 128], ident[:H, :H])
                    vec.tensor_copy(out=xbarT[:, k, b * H : (b + 1) * H], in_=pst)

            for b in range(BPC):
                for t in range(5):
                    if (b, t) not in vdone:
                        v_chunk(b, t)

            # --- branch3 sampled = wtri @ patch ---
            samp = smA.tile([M, BPC, C], F32)
            for b in range(BPC):
                ps = ppS.tile([M, C], F32, tag="psm")
                for lo, sz in N768:
                    for t in range(4):
                        pe.matmul(
                            ps[:, lo : lo + sz],
                            wtri[:, t, b * M : (b + 1) * M],
                            xg[:, b, t, lo : lo + sz],
                            start=(t == 0),
                            stop=(t == 3),
                        )
                vec.tensor_copy(out=samp[:, b], in_=ps)

            for b in range(BPC):
                for k in range(KC):
                    pst = ppS.tile([128, M], F32, tag="psm")
                    pe.transpose(
                        pst, samp[:, b, k * 128 : (k + 1) * 128], ident[:M, :M]
                    )
                    vec.tensor_copy(out=sampT[:, k, b * M : (b + 1) * M], in_=pst)

            wdkv0 = wload(wpool, "w_dbakvT", cols=C, colofs=0, name="wdkv0")
            wdkv1 = wload(wpool, "w_dbakvT", cols=C, colofs=C, name="wdkv1")
            wdbao = wload(wpool, "w_dbaoT")
            wg2 = wload(wpool, "w_g2T")
            wg3 = wload(wpool, "w_g3T")

            for m in range(12):
                w = wdkv0 if m < KC else wdkv1
                mo = m % KC
                ps = ppS.tile([128, BPC * M], F32, tag="psm")
                for k in range(KC):
                    pe.matmul(
                        ps,
                        w[:, k, mo * 128 : (mo + 1) * 128],
                        sampT[:, k, :],
                        start=(k == 0),
                        stop=(k == KC - 1),
                    )
                if m < KC:
                    vec.tensor_copy(out=kvdba[:, m], in_=ps)
                else:
                    vec.tensor_scalar_add(
                        kvdba[:, m], ps, bdbav[:, (m - KC) : (m - KC) + 1]
                    )

            # --- dba tiny attention (vec-heavy; ctx/cond matmuls fill PE) ---
            prod = smA.tile([128, BPC * M], BF16, name="prodb")
            prodf = smA.tile([128, BPC * M], F32, name="prodf")
            edba = smA.tile([2, KC, BPC * M], BF16)
            ddba = smA.tile([2, KC, BPC], F32)
            for kk in range(KC):
                vec.tensor_tensor(
                    out=prod.rearrange("p (b m) -> p b m", b=BPC),
                    in0=kvdba[:, kk, :].rearrange("p (b m) -> p b m", b=BPC),
                    in1=qdbaT[:, kk, :].unsqueeze(2).broadcast_to((128, BPC, M)),
                    op=ALU.mult,
                )
                pd = ppS.tile([2, BPC * M], F32, tag="psm", name="pd_sdba")
                pe.matmul(pd, hsplit, prod, start=True, stop=True)
                act.activation(edba[:, kk, :], pd, AF.Exp, scale=ISC)

            # ctx_ca: full 24-col matmul then block-select (+bias)
            for mo in range(KC):
                ps = ppS.tile([128, BPC * H], F32, tag="psm")
                for k in range(KC):
                    pe.matmul(
                        ps,
                        wcav[:, k, mo * 128 : (mo + 1) * 128],
                        xbarT[:, k, :],
                        start=(k == 0),
                        stop=(k == KC - 1),
                    )
                for hh in range(2):
                    h = 2 * mo + hh
                    vec.tensor_scalar_add(
                        ctxcaT[64 * hh : 64 * hh + 64, mo, :],
                        ps[64 * hh : 64 * hh + 64, h : BPC * H : H],
                        bcav[64 * hh : 64 * hh + 64, mo : mo + 1],
                    )

            vec.tensor_reduce(
                ddba,
                edba.rearrange("p k (b m) -> p k b m", b=BPC),
                axis=AX.X,
                op=ALU.add,
            )
            vec.reciprocal(ddba, ddba)
            for kk in range(KC):
                for b in range(BPC):
                    vec.tensor_scalar_mul(
                        edba[:, kk, b * M : (b + 1) * M],
                        edba[:, kk, b * M : (b + 1) * M],
                        ddba[:, kk, b : b + 1],
                    )

            proj_small(wcao, ctxcaT, bcao, condT)

            ctxdf = smA.tile([128, KC, BPC], F32, name="ctxdf")
            for kk in range(KC):
                psb = ppS.tile([128, BPC * M], F32, tag="psm")
                pe.matmul(psb, splitT, edba[:, kk, :], start=True, stop=True)
                vec.tensor_tensor(
                    out=prodf, in0=kvdba[:, KC + kk, :], in1=psb, op=ALU.mult
                )
                vec.tensor_reduce(
                    ctxdf[:, kk, :],
                    prodf.rearrange("p (b m) -> p b m", b=BPC),
                    axis=AX.X,
                    op=ALU.add,
                )
            vec.tensor_copy(out=ctxdT, in_=ctxdf)

            # psg2 rides here (PE filler under dba vec work)
            rowg2 = smA.tile([BPC, C], F32, name="rowg2")
            psg2 = ppS.tile([BPC, C], F32, tag="psm", name="psg2")
            for lo, sz in N768:
                for k in range(KC):
                    pe.matmul(
                        psg2[:, lo : lo + sz],
                        condT[:, k, :],
                        wg2[:, k, lo : lo + sz],
                        start=(k == 0),
                        stop=(k == KC - 1),
                    )
            vec.tensor_copy(out=rowg2, in_=psg2)

            proj_small(wdbao, ctxdT, bdbao, dbaT)

            # bio path column: conf*(0.5*cond + 0.5*conf*dba)
            for b in range(BPC):
                vec.tensor_scalar_mul(
                    bioc[:, :, b : b + 1], dbaT[:, :, b : b + 1], confb[:, b : b + 1]
                )
            vec.tensor_tensor(out=bioc, in0=bioc, in1=condT, op=ALU.add)
            for b in range(BPC):
                vec.tensor_scalar(
                    bioc[:, :, b : b + 1],
                    bioc[:, :, b : b + 1],
                    confb[:, b : b + 1],
                    0.5,
                    ALU.mult,
                    ALU.mult,
                )

            # gate row terms: rowt = psg3*conf + psg2 + b_g
            psg3 = ppS.tile([BPC, C], F32, tag="psm", name="psg3")
            for lo, sz in N768:
                for k in range(KC):
                    pe.matmul(
                        psg3[:, lo : lo + sz],
                        dbaT[:, k, :],
                        wg3[:, k, lo : lo + sz],
                        start=(k == 0),
                        stop=(k == KC - 1),
                    )
            rowf = smA.tile([BPC, C], F32, name="rowf")
            vec.tensor_scalar_mul(rowf, psg3, confT)
            vec.tensor_tensor(out=rowf, in0=rowf, in1=rowg2, op=ALU.add)
            vec.tensor_tensor(out=rowf, in0=rowf, in1=bg, op=ALU.add)
            # transpose to column layout [c, b] for use as sigmoid bias
            for k in range(KC):
                pst = ppS.tile([128, BPC], F32, tag="psm")
                pe.transpose(pst, rowf[:, k * 128 : (k + 1) * 128], ident[:BPC, :BPC])
                vec.tensor_copy(out=rowtc[:, k, :], in_=pst)

        # ---------------- window 2: self-attention ----------------
        with tc.tile_pool(name="soPool", bufs=1) as soP:
            soT = soP.tile([128, BPC, KC, NP], BF16)
            with tc.tile_pool(name="smallB", bufs=1) as smB, tc.tile_pool(
                name="ppB", bufs=2, space="PSUM"
            ) as ppB:
                pairs = [(b, h) for b in range(BPC) for h in range(H)]

                def scores_chunk(b, h, t, etp):
                    """scores t-chunk + its exp into slot t%2 of etp[t//2]."""
                    po, mq, mk = 64 * (h % 2), h // 2, 6 + h // 2
                    pss = ppB.tile([128, 2, 512], F32, tag="psc", bufs=3)
                    for j in range(2):
                        pe.matmul(
                            pss[:TC, j, :JC],
                            qkT[po : po + 64, b, mk, t * TC : t * TC + TC],
                            qkT[po : po + 64, b, mq, j * JC : (j + 1) * JC],
                            start=True,
                            stop=True,
                        )
                    act.activation(
                        etp[t // 2][:TC, t % 2],
                        pss[:TC, :, :JC],
                        AF.Exp,
                        scale=ISC,
                    )

                def av_chunk(b, h, t, etp, psc):
                    for j in range(2):
                        pe.matmul(
                            psc[:, j, :JC],
                            vn[:TC, b, t, 65 * h : 65 * h + 65],
                            etp[t // 2][:TC, t % 2, j, :],
                            start=(t == 0),
                            stop=(t == 4),
                        )

                def finalize(b, h, psc):
                    # two quick psum->sbuf copies release the psc bank early
                    # (enables pss bufs=3); normalize runs from SBUF after.
                    po, mq = 64 * (h % 2), h // 2
                    den = smB.tile([1, NP], F32, tag="den", bufs=2, name="den")
                    vec.tensor_copy(
                        out=den.rearrange("p (j f) -> p j f", j=2),
                        in_=psc[64:65, :, :JC],
                    )
                    sc = smB.tile([64, NP], F32, tag="sc", bufs=2, name="sc")
                    vec.tensor_copy(
                        out=sc.rearrange("p (j f) -> p j f", j=2),
                        in_=psc[:64, :, :JC],
                    )
                    rec = smB.tile([1, NP], F32, tag="rec", bufs=2, name="rec")
                    vec.reciprocal_approx_fast(rec, den)
                    rbc = smB.tile([64, NP], F32, tag="rbc", bufs=2, name="rbc")
                    gp.partition_broadcast(rbc, rec)
                    vec.tensor_tensor(
                        out=soT[po : po + 64, b, mq, :],
                        in0=sc,
                        in1=rbc,
                        op=ALU.mult,
                    )

                # software pipeline: scores(i) chunks interleaved with
                # attn@v(i-1) chunks so no PE instruction ever waits on the
                # scalar-engine exp stream (stalls reset the PE clock ramp).
                prev = None  # (b, h, etp, psc)
                for bh in pairs + [None]:
                    cur = None
                    if bh is not None:
                        b, h = bh
                        psc = ppB.tile([65, 2, 512], F32, tag="pcx", bufs=1)
                        etp = [
                            smB.tile(
                                [128, 2, 2, JC], BF16, tag="expT", bufs=8, name="expT"
                            )
                            for _ in range(3)
                        ]
                        for t in range(5):
                            scores_chunk(b, h, t, etp)
                    if bh is not None:
                        cur = (b, h, etp, psc)
                    if prev is not None:
                        for t in range(5):
                            av_chunk(prev[0], prev[1], t, prev[2], prev[3])
                        finalize(prev[0], prev[1], prev[3])
                    prev = cur

            # ---------------- window 3: gate + fusion + proj ----------------
            with tc.tile_pool(name="smallC", bufs=1) as smC, tc.tile_pool(
                name="ppC", bufs=2, space="PSUM"
            ) as ppC:
                wg1 = wload(wpool, "w_g1T", eng=sync)
                wp = wload(wpool, "w_pT", eng=sync)
                fusedT = [None, None]
                for b in range(BPC):
                    fusedT[b] = smC.tile(
                        [128, KC, NP], BF16, tag=f"fusedT{b}", bufs=1, name="fusedT"
                    )
                    for mo in range(KC):
                        ps = ppC.tile([128, 2, 512], F32, tag="pgate")
                        for j in range(2):
                            for k in range(KC):
                                pe.matmul(
                                    ps[:, j, :JC],
                                    wg1[:, k, mo * 128 : (mo + 1) * 128],
                                    soT[:, b, k, j * JC : (j + 1) * JC],
                                    start=(k == 0),
                                    stop=(k == KC - 1),
                                )
                        gateT = smC.tile(
                            [128, NP], BF16, tag="gateT", bufs=2, name="gateT"
                        )
                        act.activation(
                            gateT.rearrange("p (j f) -> p j f", j=2),
                            ps[:, :, :JC],
                            AF.Sigmoid,
                            bias=rowtc[:, mo, b : b + 1],
                        )
                        vec.scalar_tensor_tensor(
                            out=fusedT[b][:, mo, :],
                            in0=soT[:, b, mo, :],
                            scalar=bioc[:, mo, b : b + 1],
                            in1=gateT,
                            op0=ALU.subtract,
                            op1=ALU.mult,
                        )
                        vec.tensor_tensor(
                            out=fusedT[b][:, mo, :],
                            in0=soT[:, b, mo, :],
                            in1=fusedT[b][:, mo, :],
                            op=ALU.subtract,
                        )
                for b in range(BPC):
                    for mo in range(KC):
                        ps = ppC.tile([128, 2, 512], F32, tag="pgate")
                        for j in range(2):
                            for k in range(KC):
                                pe.matmul(
                                    ps[:, j, :JC],
                                    wp[:, k, mo * 128 : (mo + 1) * 128],
                                    fusedT[b][:, k, j * JC : (j + 1) * JC],
                                    start=(k == 0),
                                    stop=(k == KC - 1),
                                )
                        outT = smC.tile([128, NP], F32, tag="outT", bufs=3, name="oT")
                        vec.tensor_scalar_add(
                            outT.rearrange("p (j f) -> p j f", j=2),
                            ps[:, :, :JC],
                            bp[:, mo : mo + 1],
                        )
                        eng = sync if mo % 2 == 0 else act
                        eng.dma_start(
                            out=out.ap()[b, mo * 128 : (mo + 1) * 128, :],
                            in_=outT[:, 0:NREAL],
                        )


# ====================== host side ======================


def stage_inputs(inputs):
    """Pure layout/dtype staging of the full inputs into 8 per-core in_maps."""
    f = np.float32
    bf = ml_dtypes.bfloat16
    x = np.asarray(inputs["x"], f)
    bio = np.asarray(inputs["bio_embed"], f)
    conf = np.asarray(inputs["confidence"], f)
    bco = np.asarray(inputs["base_coords"], f)
    offs = np.asarray(inputs["offsets"], f)

    W = {k: np.asarray(v, f) for k, v in inputs.items()}
    qkv_w = W["qkv_w"]
    qkv_b = W["qkv_b"]
    wv = qkv_w[2 * C :]
    w_vT = np.zeros((C, VW), f)
    b_vw = np.zeros((1, VW), f)
    for h in range(H):
        w_vT[:, 65 * h : 65 * h + 64] = wv[64 * h : 64 * h + 64].T
        b_vw[0, 65 * h : 65 * h + 64] = qkv_b[2 * C + 64 * h : 2 * C + 64 * h + 64]
        b_vw[0, 65 * h + 64] = 1.0

    ident = np.eye(128, dtype=f)
    gz, gy, gx = np.meshgrid(np.arange(G), np.arange(G), np.arange(G), indexing="ij")
    ngrid = -np.stack([gx.ravel(), gy.ravel(), gz.ravel()], axis=1).astype(f)
    splitT = np.zeros((2, 128), f)
    splitT[0, :64] = 1.0
    splitT[1, 64:] = 1.0

    # fold bio_query into ca_query: q_ca = Wcaq @ (Wbioq @ bio + b_bioq) + b_caq
    w_caq = W["ca_in_w"][:C]
    w_caq_eff = w_caq @ W["bio_query_w"]
    b_caq_eff = w_caq @ W["bio_query_b"] + W["ca_in_b"][:C]

    shared = {
        "w_qkT": qkv_w[: 2 * C].T,
        "w_vT": w_vT,
        "w_caqT": w_caq_eff.T,
        "w_cak": W["ca_in_w"][C : 2 * C],
        "w_cavT": W["ca_in_w"][2 * C :].T,
        "w_caoT": W["ca_out_w"].T,
        "w_dbaqT": W["dba_in_w"][:C].T,
        "w_dbakvT": (W["dba_in_w"][C:] @ W["sp_w"]).T,
        "w_dbaoT": W["dba_out_w"].T,
        "w_g1T": W["gate_w"][:, :C].T,
        "w_g2T": W["gate_w"][:, C : 2 * C].T,
        "w_g3T": W["gate_w"][:, 2 * C :].T,
        "w_pT": W["proj_w"].T,
        "splitT": splitT,
        "hsplit": splitT.T,
    }
    shared = {k: np.ascontiguousarray(v, bf) for k, v in shared.items()}
    b_vwz = np.broadcast_to(b_vw, (128, VW)).copy()
    b_vwz[97:] = 0.0

    b_stack = np.stack(
        [
            b_caq_eff,
            W["ca_in_b"][2 * C :],
            W["ca_out_b"],
            W["dba_in_b"][:C],
            W["dba_in_w"][2 * C :] @ W["sp_b"] + W["dba_in_b"][2 * C :],
            W["dba_out_b"],
            np.zeros(C, np.float32),
            W["proj_b"],
        ]
    )
    shared_f32 = {
        "b_qk": qkv_b[: 2 * C],
        "b_vw": b_vw,
        "b_vwz": b_vwz,
        "b_stack": b_stack,
        "b_g": np.broadcast_to(W["gate_b"].reshape(1, C), (BPC, C)),
        "ident": ident,
        "ngrid": ngrid,
    }
    shared.update(
        {k: np.ascontiguousarray(v, f) for k, v in shared_f32.items()}
    )
    shared["bcoordsT"] = np.ascontiguousarray(
        np.broadcast_to(bco.T[:, None, :], (3, BPC, M)).reshape(1, 3, BPC * M), f
    )

    xpad = np.zeros((B, NP, C), f)
    xpad[:, :NREAL] = x

    in_maps = []
    for c in range(NCORES):
        sl = slice(c * BPC, (c + 1) * BPC)
        m = dict(shared)
        m["xn"] = np.ascontiguousarray(xpad[sl], bf)
        m["xT"] = np.ascontiguousarray(xpad[sl].transpose(0, 2, 1), bf)
        m["bioT"] = np.ascontiguousarray(bio[sl].T, bf)
        m["confb"] = np.ascontiguousarray(
            np.broadcast_to(conf[sl].reshape(1, BPC), (128, BPC)), f
        )
        m["confT"] = np.ascontiguousarray(conf[sl].reshape(BPC, 1), f)
        m["offsT"] = np.ascontiguousarray(
            offs[sl].transpose(2, 0, 1).reshape(1, 3, BPC * M), f
        )
        in_maps.append(m)
    return in_maps


_CACHE = {}


def get_nc():
    if "nc" not in _CACHE:
        nc = bacc.Bacc("TRN2", target_bir_lowering=False, debug=False)
        build(nc)
        _CACHE["nc"] = nc
    return _CACHE["nc"]


def _ensure_ntff_hook():
    """The agent image's antenv lacks axon_hooks; shim it so trace=True can
    reach the libaxon NTFF profiler (profiling only, test-harness use)."""
    import types

    try:
        import antenv.axon_hooks  # noqa: F401

        return
    except ImportError:
        pass
    mod = types.ModuleType("antenv.axon_hooks")
    state = {"h": None}
    mod.set_axon_ntff_profile_hook = lambda h: state.__setitem__("h", h)
    mod.get_axon_ntff_profile_hook = lambda: state["h"]
    sys.modules["antenv.axon_hooks"] = mod
    import antenv

    antenv.axon_hooks = mod
    try:
        from trn_agent_boot.trn_boot import _ntff_profile_via_ctypes

        hook = _ntff_profile_via_ctypes("/opt/axon/libaxon_pjrt.so")
        if hook is not None:
            mod.set_axon_ntff_profile_hook(hook)
    except Exception:
        pass


def kernel(**inputs):
    trace = bool(int(os.environ.get("KERNEL_TRACE", "0")))
    if trace:
        _ensure_ntff_hook()
    nc = get_nc()
    in_maps = stage_inputs(inputs)
    res = bass_utils.run_bass_kernel_spmd(
        nc,
        in_maps,
        core_ids=list(range(NCORES)),
        trace=trace,
    )
    _CACHE["last_result"] = res
    outT = np.stack([res.results[c]["outT"] for c in range(NCORES)])
    out = outT.reshape(B, C, N).transpose(0, 2, 1)
    return np.ascontiguousarray(out, dtype=np.float32)


# revision 69
# speedup vs baseline: 1.3189x; 1.0490x over previous
"""Trainium2 Bass kernel for BiomarkerConditionedAttention.

Sharding: pure data-parallel over batch B=16 across 8 cores (2 batches/core).

v2 layout strategy (per core; "T" = feature-on-partitions):
  - all matmul operands bf16 (PSUM accum f32); error budget 2e-2 allows it.
  - tokens padded 513 -> 520 on host (zero pad): every 513-free matmul
    becomes 2x260 PSUM-bank chunks - no 1-wide companion matmuls.
    Padded KEYS are neutralized by zeroing their vn rows (v and the
    ones-column), so they contribute 0 to both context and denominator.
    Padded QUERY columns produce garbage that is sliced off at the end.
  - branch1: q/k channel-major; v token-major widened with a per-head
    ones-column so attn@v also emits the softmax denominator row.
  - branch2 cross-attn via rank-1 algebra; bio_query projection folded
    into ca_query on host (W_caq_eff = W_caq @ W_bioq).
  - qk-trick and ca-value use masked/full-then-select batching to cut
    tiny-matmul count.
  - branch3 grid_sample as dense matmul with on-device trilinear weights.
  - gate: per-token part contracts only standard_out; conditioned/dba
    slabs are per-batch row terms added via a rank-1 selector matmul.
"""

import os
import sys

sys.path.insert(0, "/opt/trn_rl_repo")

import ml_dtypes
import numpy as np

import concourse.bass as bass
import concourse.mybir as mybir
import concourse.tile as tile
from concourse import bacc, bass_utils

F32 = mybir.dt.float32
BF16 = mybir.dt.bfloat16
FP8 = mybir.dt.float8e4
DR = mybir.MatmulPerfMode.DoubleRow
AF = mybir.ActivationFunctionType
ALU = mybir.AluOpType
AX = mybir.AxisListType

B, N, C, H, M, G = 16, 513, 768, 12, 5, 8
HD = C // H  # 64
ISC = float(1.0 / np.sqrt(HD))
NCORES = 8
BPC = B // NCORES  # 2
KC = C // 128  # 6
NP = 520  # padded token count
JC = 260  # free-dim chunk (2 per 520)
TC = 104  # token chunk (partition dim), 5 per 520
NREAL = 513
VW = H * (HD + 1)  # 780
N768 = [(0, 512), (512, 256)]
NVW = [(0, 512), (512, 268)]


def build(nc: bass.Bass):
    dram = {}

    def din(name, shape, dt=BF16):
        dram[name] = nc.dram_tensor(name, list(shape), dt, kind="ExternalInput")

    din("xT", (BPC, C, NP))
    din("xn", (BPC, NP, C))
    din("bioT", (C, BPC))
    din("confT", (BPC, 1), F32)
    din("offsT", (1, 3, BPC * M), F32)
    din("bcoordsT", (1, 3, BPC * M), F32)
    din("w_qkT", (C, 2 * C))
    din("w_vT", (C, VW))
    din("w_caqT", (C, C))
    din("w_cak", (C, C))
    din("w_cavT", (C, C))
    din("w_caoT", (C, C))
    din("w_dbaqT", (C, C))
    din("w_dbakvT", (C, 2 * C))  # sp_w folded in on host
    din("w_dbaoT", (C, C))
    din("w_g1T", (C, C))
    din("w_g2T", (C, C))
    din("w_g3T", (C, C))
    din("w_pT", (C, C))
    # merged 128-partition f32 consts: ident | ngrid | confb | bqk | bstack
    # | bvw | bvwz  (fewer DMA issues -> faster warmup)
    din("cf32", (128, 1762), F32)
    din("b_g", (BPC, C), F32)
    din("splitT", (2, 128))
    din("hsplit", (128, 2))

    out = nc.dram_tensor("outT", [BPC, C, N], F32, kind="ExternalOutput")

    with tile.TileContext(nc) as tc:
        emit(nc, tc, dram, out)
    nc.compile()
    return dram, out


def emit(nc, tc, dram, out):
    sync, vec, act, gp, pe = nc.sync, nc.vector, nc.scalar, nc.gpsimd, nc.tensor

    def wload(pool, wname, cols=C, colofs=0, name=None, eng=sync):
        t = pool.tile([128, KC, cols], BF16, tag="wbig", name=name or f"w_{wname}")
        src = dram[wname].ap()
        if cols != src.shape[1] or colofs:
            src = src[:, colofs : colofs + cols]
        eng.dma_start(out=t, in_=src.rearrange("(k p) m -> p k m", p=128))
        return t

    with tc.tile_pool(name="consts", bufs=1) as consts, tc.tile_pool(
        name="persist", bufs=1
    ) as persist, tc.tile_pool(name="wpool", bufs=4) as wpool:
        # ---------- big input DMAs first (earliest PE start) ----------
        with tc.tile_pool(name="xpool", bufs=1) as xpool, tc.tile_pool(
            name="smallA", bufs=1
        ) as smA, tc.tile_pool(name="ppA", bufs=2, space="PSUM") as ppA, tc.tile_pool(
            name="ppS", bufs=2, space="PSUM"
        ) as ppS:
            xT = xpool.tile([128, BPC, KC, NP], BF16)
            sync.dma_start(
                out=xT[:, 0, 0:3],
                in_=dram["xT"].ap()[0][0:384].rearrange("(k p) n -> p k n", p=128),
            )
            wqk = [
                wload(wpool, "w_qkT", cols=384, colofs=384 * i, name=f"wqk{i}", eng=act)
                for i in range(4)
            ]
            sync.dma_start(
                out=xT[:, 0, 3:6],
                in_=dram["xT"].ap()[0][384:768].rearrange("(k p) n -> p k n", p=128),
            )
            sync.dma_start(
                out=xT[:, 1],
                in_=dram["xT"].ap()[1].rearrange("(k p) n -> p k n", p=128),
            )

            # ---------- small consts on other queues ----------
            # offsets/coords first: the trilinear vec chain waits on them
            offsT0 = consts.tile([1, 3, BPC * M], F32, name="offsT0")
            gp.dma_start(out=offsT0, in_=dram["offsT"].ap())
            baseT0 = consts.tile([1, 3, BPC * M], F32, name="baseT0")
            gp.dma_start(out=baseT0, in_=dram["bcoordsT"].ap())
            cf32 = consts.tile([128, 1762], F32)
            gp.dma_start(out=cf32, in_=dram["cf32"].ap())
            ident = cf32[:, 0:128]
            ngrid = cf32[:, 128:140].rearrange("p (t d) -> p t d", t=4)
            confb = cf32[:, 140:142]
            bqk = cf32[:, 142:154]
            bstack = cf32[:, 154:202].rearrange("p (s k) -> p s k", s=8)
            bvw = cf32[:, 202:982]
            bvwz = cf32[:, 982:1762]
            bcaq, bcav, bcao = bstack[:, 0], bstack[:, 1], bstack[:, 2]
            bdbaq, bdbav, bdbao = bstack[:, 3], bstack[:, 4], bstack[:, 5]
            bsp, bp = bstack[:, 6], bstack[:, 7]
            bioT = consts.tile([128, KC, BPC], BF16)
            gp.dma_start(
                out=bioT, in_=dram["bioT"].ap().rearrange("(k p) b -> p k b", p=128)
            )
            confT = consts.tile([BPC, 1], F32)
            gp.dma_start(out=confT, in_=dram["confT"].ap())
            splitT = consts.tile([2, 128], BF16)
            act.dma_start(out=splitT, in_=dram["splitT"].ap())
            hsplit = consts.tile([128, 2], BF16)
            act.dma_start(out=hsplit, in_=dram["hsplit"].ap())
            bg = consts.tile([BPC, C], F32)
            act.dma_start(out=bg, in_=dram["b_g"].ap())

            # persistent activations / small intermediates
            qkT = persist.tile([128, BPC, 12, NP], BF16)
            vn = persist.tile([128, BPC, 5, VW], BF16)
            pixb = persist.tile([128, 3, BPC * M], F32)
            wtri = persist.tile([128, 4, BPC * M], BF16)
            sampT = persist.tile([128, KC, BPC * M], BF16)
            kvdba = persist.tile([128, 12, BPC * M], F32)
            qcaT = persist.tile([128, KC, BPC], BF16)
            qdbaT = persist.tile([128, KC, BPC], F32)
            qmask = persist.tile([128, KC, H * BPC], BF16)
            qkc = persist.tile([128, KC, H, BPC], BF16)
            xbarT = persist.tile([128, KC, BPC * H], BF16)
            ctxcaT = persist.tile([128, KC, BPC], BF16)
            condT = persist.tile([128, KC, BPC], BF16)
            ctxdT = persist.tile([128, KC, BPC], BF16)
            dbaT = persist.tile([128, KC, BPC], BF16)
            bioc = persist.tile([128, KC, BPC], F32)
            rowtc = persist.tile([128, KC, BPC], F32)

            xg = xpool.tile([128, BPC, 4, C], BF16)
            xa = xpool.tile([1, BPC, C], BF16)
            act.dma_start(
                out=xa, in_=dram["xn"].ap()[:, 0:1, :].rearrange("b o c -> o b c")
            )
            for b in range(BPC):
                act.dma_start(
                    out=xg[:, b],
                    in_=dram["xn"]
                    .ap()[b, 1:513, :]
                    .rearrange("(t p) c -> p t c", p=128),
                )

            # --- trilinear hat weights (vector/scalar engines only) ---
            pixT = smA.tile([1, 3, BPC * M], F32)
            vec.tensor_tensor(out=pixT, in0=offsT0, in1=baseT0, op=ALU.add)
            vec.tensor_scalar(pixT, pixT, -1.0, 1.0, ALU.max, ALU.min)
            vec.tensor_scalar(pixT, pixT, 3.5, 3.5, ALU.mult, ALU.add)
            for d in range(3):
                gp.partition_broadcast(pixb[:, d], pixT[:, d, :])
            wd = smA.tile([128, 3, BPC * M], F32)
            wtmp = smA.tile([128, BPC * M], F32)
            for t in range(4):
                for d in range(3):
                    act.activation(
                        wd[:, d], pixb[:, d], AF.Abs, bias=ngrid[:, t, d : d + 1]
                    )
                    act.activation(wd[:, d], wd[:, d], AF.Relu, bias=1.0, scale=-1.0)
                vec.tensor_tensor(out=wtmp, in0=wd[:, 0], in1=wd[:, 1], op=ALU.mult)
                vec.tensor_tensor(
                    out=wtri[:, t], in0=wtmp, in1=wd[:, 2], op=ALU.mult
                )

            # --- P1: q/k projections, channel-major, free 2x260 ---
            for b in range(BPC):
                for m in range(12):
                    w = wqk[m // 3]
                    mo = m % 3
                    ps = ppA.tile([128, 2, 512], F32, tag="pbig")
                    for j in range(2):
                        for k in range(KC):
                            pe.matmul(
                                ps[:, j, :JC],
                                w[:, k, mo * 128 : (mo + 1) * 128],
                                xT[:, b, k, j * JC : (j + 1) * JC],
                                start=(k == 0),
                                stop=(k == KC - 1),
                            )
                    vec.tensor_scalar_add(
                        qkT[:, b, m, :].rearrange("p (j f) -> p j f", j=2),
                        ps[:, :, :JC],
                        bqk[:, m : m + 1],
                    )

            # weights for later phases (deep prefetch, spread queues)
            wva = wpool.tile([128, KC, 512], BF16, tag="wbig", name="wva")
            sync.dma_start(
                out=wva,
                in_=dram["w_vT"].ap()[:, 0:512].rearrange("(k p) m -> p k m", p=128),
            )
            wvb = wpool.tile([128, KC, 268], BF16, tag="wbig", name="wvb")
            sync.dma_start(
                out=wvb,
                in_=dram["w_vT"].ap()[:, 512:780].rearrange("(k p) m -> p k m", p=128),
            )
            wcaq = wload(wpool, "w_caqT")
            wdbaq = wload(wpool, "w_dbaqT")
            wcak = wload(wpool, "w_cak")

            # --- bio-chain stage 1: q_ca (folded) and q_dba ---
            def proj_small(w, rhs_tile, bias, o):
                nf = rhs_tile.shape[-1]
                for mo in range(KC):
                    ps = ppS.tile([128, nf], F32, tag="psm", name="ps_proj")
                    for k in range(KC):
                        pe.matmul(
                            ps,
                            w[:, k, mo * 128 : (mo + 1) * 128],
                            rhs_tile[:, k, :],
                            start=(k == 0),
                            stop=(k == KC - 1),
                        )
                    if bias is None:
                        vec.tensor_copy(out=o[:, mo], in_=ps)
                    else:
                        vec.tensor_scalar_add(o[:, mo], ps, bias[:, mo : mo + 1])

            proj_small(wcaq, bioT, bcaq, qcaT)
            proj_small(wdbaq, bioT, bdbaq, qdbaT)

            # qk-trick via masked q_ca: qmask[p,kk,(h,b)] = qca[p,kk,b] iff
            # h == 2*kk + p//64 else 0; then qkc = sum_kk wcak_kk^T @ qmask_kk
            vec.memset(qmask, 0.0)
            for kk in range(KC):
                for hh in range(2):
                    h = 2 * kk + hh
                    vec.tensor_copy(
                        out=qmask[64 * hh : 64 * hh + 64, kk, h * BPC : (h + 1) * BPC],
                        in_=qcaT[64 * hh : 64 * hh + 64, kk, :],
                    )
            # filler: v projection chunk (b0,t0) while qmask vec ops run
            vdone = set()

            def v_chunk(b, t):
                vdone.add((b, t))
                ps = ppA.tile([128, VW], F32, tag="pbig")
                for (lo, sz), wv in ((NVW[0], wva), (NVW[1], wvb)):
                    for k in range(KC):
                        pe.matmul(
                            ps[:TC, lo : lo + sz],
                            xT[:, b, k, t * TC : t * TC + TC],
                            wv[:, k, :sz],
                            start=(k == 0),
                            stop=(k == KC - 1),
                        )
                # t=4: rows 97..103 are padded tokens (x=0 -> psum 0); bvwz
                # has zero bias there so vn pad rows stay exactly zero.
                bias = bvw if t < 4 else bvwz
                vec.tensor_tensor(
                    out=vn[:TC, b, t, :], in0=ps[:TC], in1=bias[:TC], op=ALU.add
                )

            v_chunk(0, 0)

            for mo in range(KC):
                ps = ppS.tile([128, H * BPC], F32, tag="psm")
                for kk in range(KC):
                    pe.matmul(
                        ps,
                        wcak[:, kk, mo * 128 : (mo + 1) * 128],
                        qmask[:, kk, :],
                        start=(kk == 0),
                        stop=(kk == KC - 1),
                    )
                vec.tensor_copy(out=qkc[:, mo], in_=ps.rearrange("p (h b) -> p h b", b=BPC))

            # --- CA scores (keys 0..512 real; pad cols never exp'd) ---
            caps = []
            for b in range(BPC):
                ps = ppS.tile([H, 2, 512], F32, tag="psm", name=f"caps{b}")
                for j in range(2):
                    for k in range(KC):
                        pe.matmul(
                            ps[:, j, :JC],
                            qkc[:, k, :, b],
                            xT[:, b, k, j * JC : (j + 1) * JC],
                            start=(k == 0),
                            stop=(k == KC - 1),
                        )
                caps.append(ps)
            v_chunk(0, 1)
            v_chunk(0, 2)

            wcav = wload(wpool, "w_cavT")
            wcao = wload(wpool, "w_caoT")

            # --- CA softmax + xbar per batch, v-chunks as PE filler ---
            for b in range(BPC):
                ps = caps[b]
                attn = smA.tile([H, NP], F32, tag="attnca", bufs=2, name="attnca")
                den = smA.tile([H, 2], F32, tag="denca", bufs=2, name="denca")
                act.activation(
                    attn[:, 0:JC], ps[:, 0, :JC], AF.Exp, scale=ISC,
                    accum_out=den[:, 0:1],
                )
                act.activation(
                    attn[:, JC:NREAL], ps[:, 1, 0 : NREAL - JC], AF.Exp, scale=ISC,
                    accum_out=den[:, 1:2],
                )
                vec.tensor_tensor(
                    out=den[:, 0:1], in0=den[:, 0:1], in1=den[:, 1:2], op=ALU.add
                )
                vec.reciprocal(den[:, 0:1], den[:, 0:1])
                vec.tensor_scalar_mul(attn[:, :NREAL], attn[:, :NREAL], den[:, 0:1])
                if b == 0:
                    v_chunk(0, 3)
                    v_chunk(0, 4)
                else:
                    v_chunk(1, 0)
                    v_chunk(1, 1)
                attnT = smA.tile([128, 5, H], BF16, tag="attnT", bufs=2, name="attnT")
                pst0 = ppS.tile([1, H], F32, tag="psm")
                pe.transpose(pst0, attn[:, 0:1], ident[:H, :H])
                vec.tensor_copy(out=attnT[0:1, 0, :], in_=pst0)
                for t in range(4):
                    pst = ppS.tile([128, H], F32, tag="psm")
                    pe.transpose(
                        pst, attn[:, 1 + 128 * t : 1 + 128 * (t + 1)], ident[:H, :H]
                    )
                    vec.tensor_copy(out=attnT[:, 1 + t, :], in_=pst)
                psx = ppS.tile([H, C], F32, tag="psm")
                for lo, sz in N768:
                    pe.matmul(
                        psx[:, lo : lo + sz],
                        attnT[0:1, 0, :],
                        xa[:, b, lo : lo + sz],
                        start=True,
                        stop=False,
                    )
                    for t in range(4):
                        pe.matmul(
                            psx[:, lo : lo + sz],
                            attnT[:, 1 + t, :],
                            xg[:, b, t, lo : lo + sz],
                            start=False,
                            stop=(t == 3),
                        )
                xbar = smA.tile([H, C], F32, tag="xbarca", bufs=2, name="xbarca")
                vec.tensor_copy(out=xbar, in_=psx)
                for k in range(KC):
                    pst = ppS.tile([128, H], F32, tag="psm")
                    pe.transpose(pst, xbar[:, k * 128 : (k + 1) * 128], ident[:H, :H])
                    vec.tensor_copy(out=xbarT[:, k, b * H : (b + 1) * H], in_=pst)

            for b in range(BPC):
                for t in range(5):
                    if (b, t) not in vdone:
                        v_chunk(b, t)

            # --- branch3 sampled = wtri @ patch ---
            samp = smA.tile([M, BPC, C], F32)
            for b in range(BPC):
                ps = ppS.tile([M, C], F32, tag="psm")
                for lo, sz in N768:
                    for t in range(4):
                        pe.matmul(
                            ps[:, lo : lo + sz],
                            wtri[:, t, b * M : (b + 1) * M],
                            xg[:, b, t, lo : lo + sz],
                            start=(t == 0),
                            stop=(t == 3),
                        )
                vec.tensor_copy(out=samp[:, b], in_=ps)

            for b in range(BPC):
                for k in range(KC):
                    pst = ppS.tile([128, M], F32, tag="psm")
                    pe.transpose(
                        pst, samp[:, b, k * 128 : (k + 1) * 128], ident[:M, :M]
                    )
                    vec.tensor_copy(out=sampT[:, k, b * M : (b + 1) * M], in_=pst)

            wdkv0 = wload(wpool, "w_dbakvT", cols=C, colofs=0, name="wdkv0")
            wdkv1 = wload(wpool, "w_dbakvT", cols=C, colofs=C, name="wdkv1")
            wdbao = wload(wpool, "w_dbaoT")
            wg2 = wload(wpool, "w_g2T")
            wg3 = wload(wpool, "w_g3T")

            for m in range(12):
                w = wdkv0 if m < KC else wdkv1
                mo = m % KC
                ps = ppS.tile([128, BPC * M], F32, tag="psm")
                for k in range(KC):
                    pe.matmul(
                        ps,
                        w[:, k, mo * 128 : (mo + 1) * 128],
                        sampT[:, k, :],
                        start=(k == 0),
                        stop=(k == KC - 1),
                    )
                if m < KC:
                    vec.tensor_copy(out=kvdba[:, m], in_=ps)
                else:
                    vec.tensor_scalar_add(
                        kvdba[:, m], ps, bdbav[:, (m - KC) : (m - KC) + 1]
                    )

            # --- dba tiny attention (vec-heavy; ctx/cond matmuls fill PE) ---
            prod = smA.tile([128, BPC * M], BF16, name="prodb")
            prodf = smA.tile([128, BPC * M], F32, name="prodf")
            edba = smA.tile([2, KC, BPC * M], BF16)
            ddba = smA.tile([2, KC, BPC], F32)
            for kk in range(KC):
                vec.tensor_tensor(
                    out=prod.rearrange("p (b m) -> p b m", b=BPC),
                    in0=kvdba[:, kk, :].rearrange("p (b m) -> p b m", b=BPC),
                    in1=qdbaT[:, kk, :].unsqueeze(2).broadcast_to((128, BPC, M)),
                    op=ALU.mult,
                )
                pd = ppS.tile([2, BPC * M], F32, tag="psm", name="pd_sdba")
                pe.matmul(pd, hsplit, prod, start=True, stop=True)
                act.activation(edba[:, kk, :], pd, AF.Exp, scale=ISC)

            # ctx_ca: full 24-col matmul then block-select (+bias)
            for mo in range(KC):
                ps = ppS.tile([128, BPC * H], F32, tag="psm")
                for k in range(KC):
                    pe.matmul(
                        ps,
                        wcav[:, k, mo * 128 : (mo + 1) * 128],
                        xbarT[:, k, :],
                        start=(k == 0),
                        stop=(k == KC - 1),
                    )
                for hh in range(2):
                    h = 2 * mo + hh
                    vec.tensor_scalar_add(
                        ctxcaT[64 * hh : 64 * hh + 64, mo, :],
                        ps[64 * hh : 64 * hh + 64, h : BPC * H : H],
                        bcav[64 * hh : 64 * hh + 64, mo : mo + 1],
                    )

            vec.tensor_reduce(
                ddba,
                edba.rearrange("p k (b m) -> p k b m", b=BPC),
                axis=AX.X,
                op=ALU.add,
            )
            vec.reciprocal(ddba, ddba)
            for kk in range(KC):
                for b in range(BPC):
                    vec.tensor_scalar_mul(
                        edba[:, kk, b * M : (b + 1) * M],
                        edba[:, kk, b * M : (b + 1) * M],
                        ddba[:, kk, b : b + 1],
                    )

            proj_small(wcao, ctxcaT, bcao, condT)

            ctxdf = smA.tile([128, KC, BPC], F32, name="ctxdf")
            for kk in range(KC):
                psb = ppS.tile([128, BPC * M], F32, tag="psm")
                pe.matmul(psb, splitT, edba[:, kk, :], start=True, stop=True)
                vec.tensor_tensor(
                    out=prodf, in0=kvdba[:, KC + kk, :], in1=psb, op=ALU.mult
                )
                vec.tensor_reduce(
                    ctxdf[:, kk, :],
                    prodf.rearrange("p (b m) -> p b m", b=BPC),
                    axis=AX.X,
                    op=ALU.add,
                )
            vec.tensor_copy(out=ctxdT, in_=ctxdf)

            # psg2 rides here (PE filler under dba vec work)
            rowg2 = smA.tile([BPC, C], F32, name="rowg2")
            psg2 = ppS.tile([BPC, C], F32, tag="psm", name="psg2")
            for lo, sz in N768:
                for k in range(KC):
                    pe.matmul(
                        psg2[:, lo : lo + sz],
                        condT[:, k, :],
                        wg2[:, k, lo : lo + sz],
                        start=(k == 0),
                        stop=(k == KC - 1),
                    )
            vec.tensor_copy(out=rowg2, in_=psg2)

            proj_small(wdbao, ctxdT, bdbao, dbaT)

            # bio path column: conf*(0.5*cond + 0.5*conf*dba)
            for b in range(BPC):
                vec.tensor_scalar_mul(
                    bioc[:, :, b : b + 1], dbaT[:, :, b : b + 1], confb[:, b : b + 1]
                )
            vec.tensor_tensor(out=bioc, in0=bioc, in1=condT, op=ALU.add)
            for b in range(BPC):
                vec.tensor_scalar(
                    bioc[:, :, b : b + 1],
                    bioc[:, :, b : b + 1],
                    confb[:, b : b + 1],
                    0.5,
                    ALU.mult,
                    ALU.mult,
                )

            # gate row terms: rowt = psg3*conf + psg2 + b_g
            psg3 = ppS.tile([BPC, C], F32, tag="psm", name="psg3")
            for lo, sz in N768:
                for k in range(KC):
                    pe.matmul(
                        psg3[:, lo : lo + sz],
                        dbaT[:, k, :],
                        wg3[:, k, lo : lo + sz],
                        start=(k == 0),
                        stop=(k == KC - 1),
                    )
            rowf = smA.tile([BPC, C], F32, name="rowf")
            vec.tensor_scalar_mul(rowf, psg3, confT)
            vec.tensor_tensor(out=rowf, in0=rowf, in1=rowg2, op=ALU.add)
            vec.tensor_tensor(out=rowf, in0=rowf, in1=bg, op=ALU.add)
            # transpose to column layout [c, b] for use as sigmoid bias
            for k in range(KC):
                pst = ppS.tile([128, BPC], F32, tag="psm")
                pe.transpose(pst, rowf[:, k * 128 : (k + 1) * 128], ident[:BPC, :BPC])
                vec.tensor_copy(out=rowtc[:, k, :], in_=pst)

        # ---------------- window 2: self-attention ----------------
        with tc.tile_pool(name="soPool", bufs=1) as soP:
            soT = soP.tile([128, BPC, KC, NP], BF16)
            with tc.tile_pool(name="smallB", bufs=1) as smB, tc.tile_pool(
                name="ppB", bufs=2, space="PSUM"
            ) as ppB:
                pairs = [(b, h) for b in range(BPC) for h in range(H)]

                def scores_chunk(b, h, t, etp):
                    """scores t-chunk + its exp into slot t%2 of etp[t//2]."""
                    po, mq, mk = 64 * (h % 2), h // 2, 6 + h // 2
                    pss = ppB.tile([128, 2, 512], F32, tag="psc", bufs=3)
                    for j in range(2):
                        pe.matmul(
                            pss[:TC, j, :JC],
                            qkT[po : po + 64, b, mk, t * TC : t * TC + TC],
                            qkT[po : po + 64, b, mq, j * JC : (j + 1) * JC],
                            start=True,
                            stop=True,
                        )
                    act.activation(
                        etp[t // 2][:TC, t % 2],
                        pss[:TC, :, :JC],
                        AF.Exp,
                        scale=ISC,
                    )

                def av_chunk(b, h, t, etp, psc):
                    for j in range(2):
                        pe.matmul(
                            psc[:, j, :JC],
                            vn[:TC, b, t, 65 * h : 65 * h + 65],
                            etp[t // 2][:TC, t % 2, j, :],
                            start=(t == 0),
                            stop=(t == 4),
                        )

                def finalize(b, h, psc):
                    # two quick psum->sbuf copies release the psc bank early
                    # (enables pss bufs=3); normalize runs from SBUF after.
                    po, mq = 64 * (h % 2), h // 2
                    den = smB.tile([1, NP], F32, tag="den", bufs=2, name="den")
                    vec.tensor_copy(
                        out=den.rearrange("p (j f) -> p j f", j=2),
                        in_=psc[64:65, :, :JC],
                    )
                    sc = smB.tile([64, NP], F32, tag="sc", bufs=2, name="sc")
                    vec.tensor_copy(
                        out=sc.rearrange("p (j f) -> p j f", j=2),
                        in_=psc[:64, :, :JC],
                    )
                    rec = smB.tile([1, NP], F32, tag="rec", bufs=2, name="rec")
                    vec.reciprocal_approx_fast(rec, den)
                    rbc = smB.tile([64, NP], F32, tag="rbc", bufs=2, name="rbc")
                    gp.partition_broadcast(rbc, rec)
                    vec.tensor_tensor(
                        out=soT[po : po + 64, b, mq, :],
                        in0=sc,
                        in1=rbc,
                        op=ALU.mult,
                    )

                # software pipeline: scores(i) chunks interleaved with
                # attn@v(i-1) chunks so no PE instruction ever waits on the
                # scalar-engine exp stream (stalls reset the PE clock ramp).
                prev = None  # (b, h, etp, psc)
                for bh in pairs + [None]:
                    cur = None
                    if bh is not None:
                        b, h = bh
                        psc = ppB.tile([65, 2, 512], F32, tag="pcx", bufs=1)
                        etp = [
                            smB.tile(
                                [128, 2, 2, JC], BF16, tag="expT", bufs=8, name="expT"
                            )
                            for _ in range(3)
                        ]
                        for t in range(5):
                            scores_chunk(b, h, t, etp)
                    if bh is not None:
                        cur = (b, h, etp, psc)
                    if prev is not None:
                        for t in range(5):
                            av_chunk(prev[0], prev[1], t, prev[2], prev[3])
                        finalize(prev[0], prev[1], prev[3])
                    prev = cur

            # ---------------- window 3: gate + fusion + proj ----------------
            with tc.tile_pool(name="smallC", bufs=1) as smC, tc.tile_pool(
                name="ppC", bufs=2, space="PSUM"
            ) as ppC:
                wg1 = wload(wpool, "w_g1T", eng=sync)
                wp = wload(wpool, "w_pT", eng=sync)
                fusedT = [None, None]
                for b in range(BPC):
                    fusedT[b] = smC.tile(
                        [128, KC, NP], BF16, tag=f"fusedT{b}", bufs=1, name="fusedT"
                    )
                    for mo in range(KC):
                        ps = ppC.tile([128, 2, 512], F32, tag="pgate")
                        for j in range(2):
                            for k in range(KC):
                                pe.matmul(
                                    ps[:, j, :JC],
                                    wg1[:, k, mo * 128 : (mo + 1) * 128],
                                    soT[:, b, k, j * JC : (j + 1) * JC],
                                    start=(k == 0),
                                    stop=(k == KC - 1),
                                )
                        gateT = smC.tile(
                            [128, NP], BF16, tag="gateT", bufs=2, name="gateT"
                        )
                        act.activation(
                            gateT.rearrange("p (j f) -> p j f", j=2),
                            ps[:, :, :JC],
                            AF.Sigmoid,
                            bias=rowtc[:, mo, b : b + 1],
                        )
                        vec.scalar_tensor_tensor(
                            out=fusedT[b][:, mo, :],
                            in0=soT[:, b, mo, :],
                            scalar=bioc[:, mo, b : b + 1],
                            in1=gateT,
                            op0=ALU.subtract,
                            op1=ALU.mult,
                        )
                        vec.tensor_tensor(
                            out=fusedT[b][:, mo, :],
                            in0=soT[:, b, mo, :],
                            in1=fusedT[b][:, mo, :],
                            op=ALU.subtract,
                        )
                for b in range(BPC):
                    for mo in range(KC):
                        ps = ppC.tile([128, 2, 512], F32, tag="pgate")
                        for j in range(2):
                            for k in range(KC):
                                pe.matmul(
                                    ps[:, j, :JC],
                                    wp[:, k, mo * 128 : (mo + 1) * 128],
                                    fusedT[b][:, k, j * JC : (j + 1) * JC],
                                    start=(k == 0),
                                    stop=(k == KC - 1),
                                )
                        outT = smC.tile([128, NP], F32, tag="outT", bufs=3, name="oT")
                        vec.tensor_scalar_add(
                            outT.rearrange("p (j f) -> p j f", j=2),
                            ps[:, :, :JC],
                            bp[:, mo : mo + 1],
                        )
                        eng = sync if mo % 2 == 0 else act
                        eng.dma_start(
                            out=out.ap()[b, mo * 128 : (mo + 1) * 128, :],
                            in_=outT[:, 0:NREAL],
                        )


# ====================== host side ======================


def stage_inputs(inputs):
    """Pure layout/dtype staging of the full inputs into 8 per-core in_maps."""
    f = np.float32
    bf = ml_dtypes.bfloat16
    x = np.asarray(inputs["x"], f)
    bio = np.asarray(inputs["bio_embed"], f)
    conf = np.asarray(inputs["confidence"], f)
    bco = np.asarray(inputs["base_coords"], f)
    offs = np.asarray(inputs["offsets"], f)

    W = {k: np.asarray(v, f) for k, v in inputs.items()}
    qkv_w = W["qkv_w"]
    qkv_b = W["qkv_b"]
    wv = qkv_w[2 * C :]
    w_vT = np.zeros((C, VW), f)
    b_vw = np.zeros((1, VW), f)
    for h in range(H):
        w_vT[:, 65 * h : 65 * h + 64] = wv[64 * h : 64 * h + 64].T
        b_vw[0, 65 * h : 65 * h + 64] = qkv_b[2 * C + 64 * h : 2 * C + 64 * h + 64]
        b_vw[0, 65 * h + 64] = 1.0

    ident = np.eye(128, dtype=f)
    gz, gy, gx = np.meshgrid(np.arange(G), np.arange(G), np.arange(G), indexing="ij")
    ngrid = -np.stack([gx.ravel(), gy.ravel(), gz.ravel()], axis=1).astype(f)
    splitT = np.zeros((2, 128), f)
    splitT[0, :64] = 1.0
    splitT[1, 64:] = 1.0

    # fold bio_query into ca_query: q_ca = Wcaq @ (Wbioq @ bio + b_bioq) + b_caq
    w_caq = W["ca_in_w"][:C]
    w_caq_eff = w_caq @ W["bio_query_w"]
    b_caq_eff = w_caq @ W["bio_query_b"] + W["ca_in_b"][:C]

    shared = {
        "w_qkT": qkv_w[: 2 * C].T,
        "w_vT": w_vT,
        "w_caqT": w_caq_eff.T,
        "w_cak": W["ca_in_w"][C : 2 * C],
        "w_cavT": W["ca_in_w"][2 * C :].T,
        "w_caoT": W["ca_out_w"].T,
        "w_dbaqT": W["dba_in_w"][:C].T,
        "w_dbakvT": (W["dba_in_w"][C:] @ W["sp_w"]).T,
        "w_dbaoT": W["dba_out_w"].T,
        "w_g1T": W["gate_w"][:, :C].T,
        "w_g2T": W["gate_w"][:, C : 2 * C].T,
        "w_g3T": W["gate_w"][:, 2 * C :].T,
        "w_pT": W["proj_w"].T,
        "splitT": splitT,
        "hsplit": splitT.T,
    }
    shared = {k: np.ascontiguousarray(v, bf) for k, v in shared.items()}
    b_vwz = np.broadcast_to(b_vw, (128, VW)).copy()
    b_vwz[97:] = 0.0

    b_stack = np.stack(
        [
            b_caq_eff,
            W["ca_in_b"][2 * C :],
            W["ca_out_b"],
            W["dba_in_b"][:C],
            W["dba_in_w"][2 * C :] @ W["sp_b"] + W["dba_in_b"][2 * C :],
            W["dba_out_b"],
            np.zeros(C, np.float32),
            W["proj_b"],
        ]
    )
    # merged [128, 1762] f32 consts: ident | ngrid | confb | bqk | bstack
    # | bvw | bvwz (confb is per-core; filled below)
    cf32 = np.zeros((128, 1762), f)
    cf32[:, 0:128] = ident
    cf32[:, 128:140] = ngrid.reshape(4, 128, 3).transpose(1, 0, 2).reshape(128, 12)
    cf32[:, 142:154] = qkv_b[: 2 * C].reshape(12, 128).T
    cf32[:, 154:202] = b_stack.reshape(8, KC, 128).transpose(2, 0, 1).reshape(128, 48)
    cf32[:, 202:982] = np.broadcast_to(b_vw, (128, VW))
    cf32[:, 982:1762] = b_vwz
    shared_f32 = {
        "b_g": np.broadcast_to(W["gate_b"].reshape(1, C), (BPC, C)),
    }
    shared.update(
        {k: np.ascontiguousarray(v, f) for k, v in shared_f32.items()}
    )
    shared["bcoordsT"] = np.ascontiguousarray(
        np.broadcast_to(bco.T[:, None, :], (3, BPC, M)).reshape(1, 3, BPC * M), f
    )

    xpad = np.zeros((B, NP, C), f)
    xpad[:, :NREAL] = x

    in_maps = []
    for c in range(NCORES):
        sl = slice(c * BPC, (c + 1) * BPC)
        m = dict(shared)
        m["xn"] = np.ascontiguousarray(xpad[sl], bf)
        m["xT"] = np.ascontiguousarray(xpad[sl].transpose(0, 2, 1), bf)
        m["bioT"] = np.ascontiguousarray(bio[sl].T, bf)
        cfc = cf32.copy()
        cfc[:, 140:142] = np.broadcast_to(conf[sl].reshape(1, BPC), (128, BPC))
        m["cf32"] = cfc
        m["confT"] = np.ascontiguousarray(conf[sl].reshape(BPC, 1), f)
        m["offsT"] = np.ascontiguousarray(
            offs[sl].transpose(2, 0, 1).reshape(1, 3, BPC * M), f
        )
        in_maps.append(m)
    return in_maps


_CACHE = {}


def get_nc():
    if "nc" not in _CACHE:
        nc = bacc.Bacc("TRN2", target_bir_lowering=False, debug=False)
        build(nc)
        _CACHE["nc"] = nc
    return _CACHE["nc"]


def _ensure_ntff_hook():
    """The agent image's antenv lacks axon_hooks; shim it so trace=True can
    reach the libaxon NTFF profiler (profiling only, test-harness use)."""
    import types

    try:
        import antenv.axon_hooks  # noqa: F401

        return
    except ImportError:
        pass
    mod = types.ModuleType("antenv.axon_hooks")
    state = {"h": None}
    mod.set_axon_ntff_profile_hook = lambda h: state.__setitem__("h", h)
    mod.get_axon_ntff_profile_hook = lambda: state["h"]
    sys.modules["antenv.axon_hooks"] = mod
    import antenv

    antenv.axon_hooks = mod
    try:
        from trn_agent_boot.trn_boot import _ntff_profile_via_ctypes

        hook = _ntff_profile_via_ctypes("/opt/axon/libaxon_pjrt.so")
        if hook is not None:
            mod.set_axon_ntff_profile_hook(hook)
    except Exception:
        pass


def kernel(**inputs):
    trace = bool(int(os.environ.get("KERNEL_TRACE", "0")))
    if trace:
        _ensure_ntff_hook()
    nc = get_nc()
    in_maps = stage_inputs(inputs)
    res = bass_utils.run_bass_kernel_spmd(
        nc,
        in_maps,
        core_ids=list(range(NCORES)),
        trace=trace,
    )
    _CACHE["last_result"] = res
    outT = np.stack([res.results[c]["outT"] for c in range(NCORES)])
    out = outT.reshape(B, C, N).transpose(0, 2, 1)
    return np.ascontiguousarray(out, dtype=np.float32)


# revision 71
# speedup vs baseline: 1.3281x; 1.0070x over previous
"""Trainium2 Bass kernel for BiomarkerConditionedAttention.

Sharding: pure data-parallel over batch B=16 across 8 cores (2 batches/core).

v2 layout strategy (per core; "T" = feature-on-partitions):
  - all matmul operands bf16 (PSUM accum f32); error budget 2e-2 allows it.
  - tokens padded 513 -> 520 on host (zero pad): every 513-free matmul
    becomes 2x260 PSUM-bank chunks - no 1-wide companion matmuls.
    Padded KEYS are neutralized by zeroing their vn rows (v and the
    ones-column), so they contribute 0 to both context and denominator.
    Padded QUERY columns produce garbage that is sliced off at the end.
  - branch1: q/k channel-major; v token-major widened with a per-head
    ones-column so attn@v also emits the softmax denominator row.
  - branch2 cross-attn via rank-1 algebra; bio_query projection folded
    into ca_query on host (W_caq_eff = W_caq @ W_bioq).
  - qk-trick and ca-value use masked/full-then-select batching to cut
    tiny-matmul count.
  - branch3 grid_sample as dense matmul with on-device trilinear weights.
  - gate: per-token part contracts only standard_out; conditioned/dba
    slabs are per-batch row terms added via a rank-1 selector matmul.
"""

import os
import sys

sys.path.insert(0, "/opt/trn_rl_repo")

import ml_dtypes
import numpy as np

import concourse.bass as bass
import concourse.mybir as mybir
import concourse.tile as tile
from concourse import bacc, bass_utils

F32 = mybir.dt.float32
BF16 = mybir.dt.bfloat16
FP8 = mybir.dt.float8e4
DR = mybir.MatmulPerfMode.DoubleRow
AF = mybir.ActivationFunctionType
ALU = mybir.AluOpType
AX = mybir.AxisListType

B, N, C, H, M, G = 16, 513, 768, 12, 5, 8
HD = C // H  # 64
ISC = float(1.0 / np.sqrt(HD))
NCORES = 8
BPC = B // NCORES  # 2
KC = C // 128  # 6
NP = 520  # padded token count
JC = 260  # free-dim chunk (2 per 520)
TC = 104  # token chunk (partition dim), 5 per 520
NREAL = 513
VW = H * (HD + 1)  # 780
N768 = [(0, 512), (512, 256)]
NVW = [(0, 512), (512, 268)]


def build(nc: bass.Bass):
    dram = {}

    def din(name, shape, dt=BF16):
        dram[name] = nc.dram_tensor(name, list(shape), dt, kind="ExternalInput")

    din("xT", (BPC, C, NP))
    din("xn", (BPC, NP, C))
    din("bioT", (C, BPC))
    din("confT", (BPC, 1), F32)
    din("offsT", (1, 3, BPC * M), F32)
    din("bcoordsT", (1, 3, BPC * M), F32)
    din("w_qkT", (C, 2 * C))
    din("w_vT", (C, VW))
    din("w_caqT", (C, C))
    din("w_cak", (C, C))
    din("w_cavT", (C, C))
    din("w_caoT", (C, C))
    din("w_dbaqT", (C, C))
    din("w_dbakvT", (C, 2 * C))  # sp_w folded in on host
    din("w_dbaoT", (C, C))
    din("w_g1T", (C, C))
    din("w_g2T", (C, C))
    din("w_g3T", (C, C))
    din("w_pT", (C, C))
    # merged 128-partition f32 consts: ident | ngrid | confb | bqk | bstack
    # | bvw | bvwz  (fewer DMA issues -> faster warmup)
    din("cf32", (128, 1762), F32)
    din("b_g", (BPC, C), F32)
    din("splitT", (2, 128))
    din("hsplit", (128, 2))

    out = nc.dram_tensor("outT", [BPC, C, N], F32, kind="ExternalOutput")

    with tile.TileContext(nc) as tc:
        emit(nc, tc, dram, out)
    nc.compile()
    return dram, out


def emit(nc, tc, dram, out):
    sync, vec, act, gp, pe = nc.sync, nc.vector, nc.scalar, nc.gpsimd, nc.tensor

    def wload(pool, wname, cols=C, colofs=0, name=None, eng=sync):
        t = pool.tile([128, KC, cols], BF16, tag="wbig", name=name or f"w_{wname}")
        src = dram[wname].ap()
        if cols != src.shape[1] or colofs:
            src = src[:, colofs : colofs + cols]
        eng.dma_start(out=t, in_=src.rearrange("(k p) m -> p k m", p=128))
        return t

    with tc.tile_pool(name="consts", bufs=1) as consts, tc.tile_pool(
        name="persist", bufs=1
    ) as persist, tc.tile_pool(name="wpool", bufs=4) as wpool:
        # ---------- big input DMAs first (earliest PE start) ----------
        with tc.tile_pool(name="xpool", bufs=1) as xpool, tc.tile_pool(
            name="smallA", bufs=1
        ) as smA, tc.tile_pool(name="ppA", bufs=2, space="PSUM") as ppA, tc.tile_pool(
            name="ppS", bufs=2, space="PSUM"
        ) as ppS:
            xT = xpool.tile([128, BPC, KC, NP], BF16)
            sync.dma_start(
                out=xT[:, 0, 0:3],
                in_=dram["xT"].ap()[0][0:384].rearrange("(k p) n -> p k n", p=128),
            )
            wqk = [
                wload(wpool, "w_qkT", cols=384, colofs=384 * i, name=f"wqk{i}", eng=act)
                for i in range(4)
            ]
            sync.dma_start(
                out=xT[:, 0, 3:6],
                in_=dram["xT"].ap()[0][384:768].rearrange("(k p) n -> p k n", p=128),
            )
            sync.dma_start(
                out=xT[:, 1],
                in_=dram["xT"].ap()[1].rearrange("(k p) n -> p k n", p=128),
            )

            # ---------- small consts on other queues ----------
            # offsets/coords first: the trilinear vec chain waits on them
            offsT0 = consts.tile([1, 3, BPC * M], F32, name="offsT0")
            gp.dma_start(out=offsT0, in_=dram["offsT"].ap())
            baseT0 = consts.tile([1, 3, BPC * M], F32, name="baseT0")
            gp.dma_start(out=baseT0, in_=dram["bcoordsT"].ap())
            cf32 = consts.tile([128, 1762], F32)
            gp.dma_start(out=cf32, in_=dram["cf32"].ap())
            ident = cf32[:, 0:128]
            ngrid = cf32[:, 128:140].rearrange("p (t d) -> p t d", t=4)
            confb = cf32[:, 140:142]
            bqk = cf32[:, 142:154]
            bstack = cf32[:, 154:202].rearrange("p (s k) -> p s k", s=8)
            bvw = cf32[:, 202:982]
            bvwz = cf32[:, 982:1762]
            bcaq, bcav, bcao = bstack[:, 0], bstack[:, 1], bstack[:, 2]
            bdbaq, bdbav, bdbao = bstack[:, 3], bstack[:, 4], bstack[:, 5]
            bsp, bp = bstack[:, 6], bstack[:, 7]
            bioT = consts.tile([128, KC, BPC], BF16)
            gp.dma_start(
                out=bioT, in_=dram["bioT"].ap().rearrange("(k p) b -> p k b", p=128)
            )
            confT = consts.tile([BPC, 1], F32)
            gp.dma_start(out=confT, in_=dram["confT"].ap())
            splitT = consts.tile([2, 128], BF16)
            act.dma_start(out=splitT, in_=dram["splitT"].ap())
            hsplit = consts.tile([128, 2], BF16)
            act.dma_start(out=hsplit, in_=dram["hsplit"].ap())
            bg = consts.tile([BPC, C], F32)
            act.dma_start(out=bg, in_=dram["b_g"].ap())

            # persistent activations / small intermediates
            qkT = persist.tile([128, BPC, 12, NP], BF16)
            vn = persist.tile([128, BPC, 5, VW], BF16)
            pixb = persist.tile([128, 3, BPC * M], F32)
            wtri = persist.tile([128, 4, BPC * M], BF16)
            sampT = persist.tile([128, KC, BPC * M], BF16)
            kvdba = persist.tile([128, 12, BPC * M], F32)
            qcaT = persist.tile([128, KC, BPC], BF16)
            qdbaT = persist.tile([128, KC, BPC], F32)
            qmask = persist.tile([128, KC, H * BPC], BF16)
            qkc = persist.tile([128, KC, H, BPC], BF16)
            xbarT = persist.tile([128, KC, BPC * H], BF16)
            ctxcaT = persist.tile([128, KC, BPC], BF16)
            condT = persist.tile([128, KC, BPC], BF16)
            ctxdT = persist.tile([128, KC, BPC], BF16)
            dbaT = persist.tile([128, KC, BPC], BF16)
            bioc = persist.tile([128, KC, BPC], F32)
            rowtc = persist.tile([128, KC, BPC], F32)

            xg = xpool.tile([128, BPC, 4, C], BF16)
            xa = xpool.tile([1, BPC, C], BF16)
            act.dma_start(
                out=xa, in_=dram["xn"].ap()[:, 0:1, :].rearrange("b o c -> o b c")
            )
            for b in range(BPC):
                act.dma_start(
                    out=xg[:, b],
                    in_=dram["xn"]
                    .ap()[b, 1:513, :]
                    .rearrange("(t p) c -> p t c", p=128),
                )

            # --- trilinear hat weights (vector/scalar engines only) ---
            pixT = smA.tile([1, 3, BPC * M], F32)
            vec.tensor_tensor(out=pixT, in0=offsT0, in1=baseT0, op=ALU.add)
            vec.tensor_scalar(pixT, pixT, -1.0, 1.0, ALU.max, ALU.min)
            vec.tensor_scalar(pixT, pixT, 3.5, 3.5, ALU.mult, ALU.add)
            for d in range(3):
                gp.partition_broadcast(pixb[:, d], pixT[:, d, :])
            wd = smA.tile([128, 3, BPC * M], F32)
            wtmp = smA.tile([128, BPC * M], F32)
            for t in range(4):
                for d in range(3):
                    act.activation(
                        wd[:, d], pixb[:, d], AF.Abs, bias=ngrid[:, t, d : d + 1]
                    )
                    act.activation(wd[:, d], wd[:, d], AF.Relu, bias=1.0, scale=-1.0)
                vec.tensor_tensor(out=wtmp, in0=wd[:, 0], in1=wd[:, 1], op=ALU.mult)
                vec.tensor_tensor(
                    out=wtri[:, t], in0=wtmp, in1=wd[:, 2], op=ALU.mult
                )

            # --- P1: q/k projections, channel-major, free 2x260 ---
            for b in range(BPC):
                for m in range(12):
                    w = wqk[m // 3]
                    mo = m % 3
                    ps = ppA.tile([128, 2, 512], F32, tag="pbig")
                    for j in range(2):
                        for k in range(KC):
                            pe.matmul(
                                ps[:, j, :JC],
                                w[:, k, mo * 128 : (mo + 1) * 128],
                                xT[:, b, k, j * JC : (j + 1) * JC],
                                start=(k == 0),
                                stop=(k == KC - 1),
                            )
                    vec.tensor_scalar_add(
                        qkT[:, b, m, :].rearrange("p (j f) -> p j f", j=2),
                        ps[:, :, :JC],
                        bqk[:, m : m + 1],
                    )

            # weights for later phases (deep prefetch, spread queues)
            wva = wpool.tile([128, KC, 512], BF16, tag="wbig", name="wva")
            sync.dma_start(
                out=wva,
                in_=dram["w_vT"].ap()[:, 0:512].rearrange("(k p) m -> p k m", p=128),
            )
            wvb = wpool.tile([128, KC, 268], BF16, tag="wbig", name="wvb")
            sync.dma_start(
                out=wvb,
                in_=dram["w_vT"].ap()[:, 512:780].rearrange("(k p) m -> p k m", p=128),
            )
            wcaq = wload(wpool, "w_caqT")
            wdbaq = wload(wpool, "w_dbaqT")
            wcak = wload(wpool, "w_cak")

            # --- bio-chain stage 1: q_ca (folded) and q_dba ---
            def proj_small(w, rhs_tile, bias, o):
                nf = rhs_tile.shape[-1]
                for mo in range(KC):
                    ps = ppS.tile([128, nf], F32, tag="psm", name="ps_proj")
                    for k in range(KC):
                        pe.matmul(
                            ps,
                            w[:, k, mo * 128 : (mo + 1) * 128],
                            rhs_tile[:, k, :],
                            start=(k == 0),
                            stop=(k == KC - 1),
                        )
                    if bias is None:
                        vec.tensor_copy(out=o[:, mo], in_=ps)
                    else:
                        vec.tensor_scalar_add(o[:, mo], ps, bias[:, mo : mo + 1])

            proj_small(wcaq, bioT, bcaq, qcaT)
            proj_small(wdbaq, bioT, bdbaq, qdbaT)

            # qk-trick via masked q_ca: qmask[p,kk,(h,b)] = qca[p,kk,b] iff
            # h == 2*kk + p//64 else 0; then qkc = sum_kk wcak_kk^T @ qmask_kk
            vec.memset(qmask, 0.0)
            for kk in range(KC):
                for hh in range(2):
                    h = 2 * kk + hh
                    vec.tensor_copy(
                        out=qmask[64 * hh : 64 * hh + 64, kk, h * BPC : (h + 1) * BPC],
                        in_=qcaT[64 * hh : 64 * hh + 64, kk, :],
                    )
            # filler: v projection chunk (b0,t0) while qmask vec ops run
            vdone = set()

            def v_chunk(b, t):
                vdone.add((b, t))
                ps = ppA.tile([128, VW], F32, tag="pbig")
                for (lo, sz), wv in ((NVW[0], wva), (NVW[1], wvb)):
                    for k in range(KC):
                        pe.matmul(
                            ps[:TC, lo : lo + sz],
                            xT[:, b, k, t * TC : t * TC + TC],
                            wv[:, k, :sz],
                            start=(k == 0),
                            stop=(k == KC - 1),
                        )
                # t=4: rows 97..103 are padded tokens (x=0 -> psum 0); bvwz
                # has zero bias there so vn pad rows stay exactly zero.
                bias = bvw if t < 4 else bvwz
                vec.tensor_tensor(
                    out=vn[:TC, b, t, :], in0=ps[:TC], in1=bias[:TC], op=ALU.add
                )

            v_chunk(0, 0)

            for mo in range(KC):
                ps = ppS.tile([128, H * BPC], F32, tag="psm")
                for kk in range(KC):
                    pe.matmul(
                        ps,
                        wcak[:, kk, mo * 128 : (mo + 1) * 128],
                        qmask[:, kk, :],
                        start=(kk == 0),
                        stop=(kk == KC - 1),
                    )
                vec.tensor_copy(out=qkc[:, mo], in_=ps.rearrange("p (h b) -> p h b", b=BPC))

            # --- CA scores (keys 0..512 real; pad cols never exp'd) ---
            caps = []
            for b in range(BPC):
                ps = ppS.tile([H, 2, 512], F32, tag="psm", name=f"caps{b}")
                for j in range(2):
                    for k in range(KC):
                        pe.matmul(
                            ps[:, j, :JC],
                            qkc[:, k, :, b],
                            xT[:, b, k, j * JC : (j + 1) * JC],
                            start=(k == 0),
                            stop=(k == KC - 1),
                        )
                caps.append(ps)
            v_chunk(0, 1)
            v_chunk(0, 2)

            wcav = wload(wpool, "w_cavT")
            wcao = wload(wpool, "w_caoT")

            # --- CA softmax + xbar per batch, v-chunks as PE filler ---
            for b in range(BPC):
                ps = caps[b]
                attn = smA.tile([H, NP], F32, tag="attnca", bufs=2, name="attnca")
                den = smA.tile([H, 2], F32, tag="denca", bufs=2, name="denca")
                act.activation(
                    attn[:, 0:JC], ps[:, 0, :JC], AF.Exp, scale=ISC,
                    accum_out=den[:, 0:1],
                )
                act.activation(
                    attn[:, JC:NREAL], ps[:, 1, 0 : NREAL - JC], AF.Exp, scale=ISC,
                    accum_out=den[:, 1:2],
                )
                vec.tensor_tensor(
                    out=den[:, 0:1], in0=den[:, 0:1], in1=den[:, 1:2], op=ALU.add
                )
                vec.reciprocal(den[:, 0:1], den[:, 0:1])
                vec.tensor_scalar_mul(attn[:, :NREAL], attn[:, :NREAL], den[:, 0:1])
                if b == 0:
                    v_chunk(0, 3)
                    v_chunk(0, 4)
                else:
                    v_chunk(1, 0)
                    v_chunk(1, 1)
                attnT = smA.tile([128, 5, H], BF16, tag="attnT", bufs=2, name="attnT")
                pst0 = ppS.tile([1, H], F32, tag="psm")
                pe.transpose(pst0, attn[:, 0:1], ident[:H, :H])
                vec.tensor_copy(out=attnT[0:1, 0, :], in_=pst0)
                for t in range(4):
                    pst = ppS.tile([128, H], F32, tag="psm")
                    pe.transpose(
                        pst, attn[:, 1 + 128 * t : 1 + 128 * (t + 1)], ident[:H, :H]
                    )
                    vec.tensor_copy(out=attnT[:, 1 + t, :], in_=pst)
                psx = ppS.tile([H, C], F32, tag="psm")
                for lo, sz in N768:
                    pe.matmul(
                        psx[:, lo : lo + sz],
                        attnT[0:1, 0, :],
                        xa[:, b, lo : lo + sz],
                        start=True,
                        stop=False,
                    )
                    for t in range(4):
                        pe.matmul(
                            psx[:, lo : lo + sz],
                            attnT[:, 1 + t, :],
                            xg[:, b, t, lo : lo + sz],
                            start=False,
                            stop=(t == 3),
                        )
                xbar = smA.tile([H, C], F32, tag="xbarca", bufs=2, name="xbarca")
                vec.tensor_copy(out=xbar, in_=psx)
                for k in range(KC):
                    pst = ppS.tile([128, H], F32, tag="psm")
                    pe.transpose(pst, xbar[:, k * 128 : (k + 1) * 128], ident[:H, :H])
                    vec.tensor_copy(out=xbarT[:, k, b * H : (b + 1) * H], in_=pst)

            for b in range(BPC):
                for t in range(5):
                    if (b, t) not in vdone:
                        v_chunk(b, t)

            # --- branch3 sampled = wtri @ patch ---
            samp = smA.tile([M, BPC, C], F32)
            for b in range(BPC):
                ps = ppS.tile([M, C], F32, tag="psm")
                for lo, sz in N768:
                    for t in range(4):
                        pe.matmul(
                            ps[:, lo : lo + sz],
                            wtri[:, t, b * M : (b + 1) * M],
                            xg[:, b, t, lo : lo + sz],
                            start=(t == 0),
                            stop=(t == 3),
                        )
                vec.tensor_copy(out=samp[:, b], in_=ps)

            for b in range(BPC):
                for k in range(KC):
                    pst = ppS.tile([128, M], F32, tag="psm")
                    pe.transpose(
                        pst, samp[:, b, k * 128 : (k + 1) * 128], ident[:M, :M]
                    )
                    vec.tensor_copy(out=sampT[:, k, b * M : (b + 1) * M], in_=pst)

            wdkv0 = wload(wpool, "w_dbakvT", cols=C, colofs=0, name="wdkv0")
            wdkv1 = wload(wpool, "w_dbakvT", cols=C, colofs=C, name="wdkv1")
            wdbao = wload(wpool, "w_dbaoT")
            wg2 = wload(wpool, "w_g2T")
            wg3 = wload(wpool, "w_g3T")

            for m in range(12):
                w = wdkv0 if m < KC else wdkv1
                mo = m % KC
                ps = ppS.tile([128, BPC * M], F32, tag="psm")
                for k in range(KC):
                    pe.matmul(
                        ps,
                        w[:, k, mo * 128 : (mo + 1) * 128],
                        sampT[:, k, :],
                        start=(k == 0),
                        stop=(k == KC - 1),
                    )
                if m < KC:
                    vec.tensor_copy(out=kvdba[:, m], in_=ps)
                else:
                    vec.tensor_scalar_add(
                        kvdba[:, m], ps, bdbav[:, (m - KC) : (m - KC) + 1]
                    )

            # --- dba tiny attention (vec-heavy; ctx/cond matmuls fill PE) ---
            prod = smA.tile([128, BPC * M], BF16, name="prodb")
            prodf = smA.tile([128, BPC * M], F32, name="prodf")
            edba = smA.tile([2, KC, BPC * M], BF16)
            ddba = smA.tile([2, KC, BPC], F32)
            for kk in range(KC):
                vec.tensor_tensor(
                    out=prod.rearrange("p (b m) -> p b m", b=BPC),
                    in0=kvdba[:, kk, :].rearrange("p (b m) -> p b m", b=BPC),
                    in1=qdbaT[:, kk, :].unsqueeze(2).broadcast_to((128, BPC, M)),
                    op=ALU.mult,
                )
                pd = ppS.tile([2, BPC * M], F32, tag="psm", name="pd_sdba")
                pe.matmul(pd, hsplit, prod, start=True, stop=True)
                act.activation(edba[:, kk, :], pd, AF.Exp, scale=ISC)

            # ctx_ca: full 24-col matmul then block-select (+bias)
            for mo in range(KC):
                ps = ppS.tile([128, BPC * H], F32, tag="psm")
                for k in range(KC):
                    pe.matmul(
                        ps,
                        wcav[:, k, mo * 128 : (mo + 1) * 128],
                        xbarT[:, k, :],
                        start=(k == 0),
                        stop=(k == KC - 1),
                    )
                for hh in range(2):
                    h = 2 * mo + hh
                    vec.tensor_scalar_add(
                        ctxcaT[64 * hh : 64 * hh + 64, mo, :],
                        ps[64 * hh : 64 * hh + 64, h : BPC * H : H],
                        bcav[64 * hh : 64 * hh + 64, mo : mo + 1],
                    )

            vec.tensor_reduce(
                ddba,
                edba.rearrange("p k (b m) -> p k b m", b=BPC),
                axis=AX.X,
                op=ALU.add,
            )
            vec.reciprocal(ddba, ddba)
            for kk in range(KC):
                for b in range(BPC):
                    vec.tensor_scalar_mul(
                        edba[:, kk, b * M : (b + 1) * M],
                        edba[:, kk, b * M : (b + 1) * M],
                        ddba[:, kk, b : b + 1],
                    )

            proj_small(wcao, ctxcaT, bcao, condT)

            ctxdf = smA.tile([128, KC, BPC], F32, name="ctxdf")
            for kk in range(KC):
                psb = ppS.tile([128, BPC * M], F32, tag="psm")
                pe.matmul(psb, splitT, edba[:, kk, :], start=True, stop=True)
                vec.tensor_tensor(
                    out=prodf, in0=kvdba[:, KC + kk, :], in1=psb, op=ALU.mult
                )
                vec.tensor_reduce(
                    ctxdf[:, kk, :],
                    prodf.rearrange("p (b m) -> p b m", b=BPC),
                    axis=AX.X,
                    op=ALU.add,
                )
            vec.tensor_copy(out=ctxdT, in_=ctxdf)

            # psg2 rides here (PE filler under dba vec work)
            rowg2 = smA.tile([BPC, C], F32, name="rowg2")
            psg2 = ppS.tile([BPC, C], F32, tag="psm", name="psg2")
            for lo, sz in N768:
                for k in range(KC):
                    pe.matmul(
                        psg2[:, lo : lo + sz],
                        condT[:, k, :],
                        wg2[:, k, lo : lo + sz],
                        start=(k == 0),
                        stop=(k == KC - 1),
                    )
            vec.tensor_copy(out=rowg2, in_=psg2)

            proj_small(wdbao, ctxdT, bdbao, dbaT)

            # bio path column: conf*(0.5*cond + 0.5*conf*dba)
            for b in range(BPC):
                vec.tensor_scalar_mul(
                    bioc[:, :, b : b + 1], dbaT[:, :, b : b + 1], confb[:, b : b + 1]
                )
            vec.tensor_tensor(out=bioc, in0=bioc, in1=condT, op=ALU.add)
            for b in range(BPC):
                vec.tensor_scalar(
                    bioc[:, :, b : b + 1],
                    bioc[:, :, b : b + 1],
                    confb[:, b : b + 1],
                    0.5,
                    ALU.mult,
                    ALU.mult,
                )

            # gate row terms: rowt = psg3*conf + psg2 + b_g
            psg3 = ppS.tile([BPC, C], F32, tag="psm", name="psg3")
            for lo, sz in N768:
                for k in range(KC):
                    pe.matmul(
                        psg3[:, lo : lo + sz],
                        dbaT[:, k, :],
                        wg3[:, k, lo : lo + sz],
                        start=(k == 0),
                        stop=(k == KC - 1),
                    )
            rowf = smA.tile([BPC, C], F32, name="rowf")
            vec.tensor_scalar_mul(rowf, psg3, confT)
            vec.tensor_tensor(out=rowf, in0=rowf, in1=rowg2, op=ALU.add)
            vec.tensor_tensor(out=rowf, in0=rowf, in1=bg, op=ALU.add)
            # transpose to column layout [c, b] for use as sigmoid bias
            for k in range(KC):
                pst = ppS.tile([128, BPC], F32, tag="psm")
                pe.transpose(pst, rowf[:, k * 128 : (k + 1) * 128], ident[:BPC, :BPC])
                vec.tensor_copy(out=rowtc[:, k, :], in_=pst)

        # ---------------- window 2: self-attention ----------------
        with tc.tile_pool(name="soPool", bufs=1) as soP:
            soT = soP.tile([128, BPC, KC, NP], BF16)
            with tc.tile_pool(name="smallB", bufs=1) as smB, tc.tile_pool(
                name="ppB", bufs=2, space="PSUM"
            ) as ppB:
                pairs = [(b, h) for b in range(BPC) for h in range(H)]

                def scores_chunk(b, h, t, etp):
                    """scores t-chunk + its exp into slot t%2 of etp[t//2]."""
                    po, mq, mk = 64 * (h % 2), h // 2, 6 + h // 2
                    pss = ppB.tile([128, 2, 512], F32, tag="psc", bufs=3)
                    for j in range(2):
                        pe.matmul(
                            pss[:TC, j, :JC],
                            qkT[po : po + 64, b, mk, t * TC : t * TC + TC],
                            qkT[po : po + 64, b, mq, j * JC : (j + 1) * JC],
                            start=True,
                            stop=True,
                        )
                    act.activation(
                        etp[t // 2][:TC, t % 2],
                        pss[:TC, :, :JC],
                        AF.Exp,
                        scale=ISC,
                    )

                def av_chunk(b, h, t, etp, psc):
                    for j in range(2):
                        pe.matmul(
                            psc[:, j, :JC],
                            vn[:TC, b, t, 65 * h : 65 * h + 65],
                            etp[t // 2][:TC, t % 2, j, :],
                            start=(t == 0),
                            stop=(t == 4),
                        )

                def finalize(b, h, psc):
                    # two quick psum->sbuf copies release the psc bank early
                    # (enables pss bufs=3); normalize runs from SBUF after.
                    po, mq = 64 * (h % 2), h // 2
                    den = smB.tile([1, NP], F32, tag="den", bufs=2, name="den")
                    vec.tensor_copy(
                        out=den.rearrange("p (j f) -> p j f", j=2),
                        in_=psc[64:65, :, :JC],
                    )
                    sc = smB.tile([64, NP], F32, tag="sc", bufs=2, name="sc")
                    vec.tensor_copy(
                        out=sc.rearrange("p (j f) -> p j f", j=2),
                        in_=psc[:64, :, :JC],
                    )
                    rec = smB.tile([1, NP], F32, tag="rec", bufs=2, name="rec")
                    vec.reciprocal_approx_fast(rec, den)
                    rbc = smB.tile([64, NP], F32, tag="rbc", bufs=2, name="rbc")
                    gp.partition_broadcast(rbc, rec)
                    vec.tensor_tensor(
                        out=soT[po : po + 64, b, mq, :],
                        in0=sc,
                        in1=rbc,
                        op=ALU.mult,
                    )

                # software pipeline: scores(i) chunks interleaved with
                # attn@v(i-1) chunks so no PE instruction ever waits on the
                # scalar-engine exp stream (stalls reset the PE clock ramp).
                prev = None  # (b, h, etp, psc)
                for bh in pairs + [None]:
                    cur = None
                    if bh is not None:
                        b, h = bh
                        psc = ppB.tile([65, 2, 512], F32, tag="pcx", bufs=1)
                        etp = [
                            smB.tile(
                                [128, 2, 2, JC], BF16, tag="expT", bufs=12, name="expT"
                            )
                            for _ in range(3)
                        ]
                        for t in range(5):
                            scores_chunk(b, h, t, etp)
                    if bh is not None:
                        cur = (b, h, etp, psc)
                    if prev is not None:
                        for t in range(5):
                            av_chunk(prev[0], prev[1], t, prev[2], prev[3])
                        finalize(prev[0], prev[1], prev[3])
                    prev = cur

            # ---------------- window 3: gate + fusion + proj ----------------
            with tc.tile_pool(name="smallC", bufs=1) as smC, tc.tile_pool(
                name="ppC", bufs=2, space="PSUM"
            ) as ppC:
                wg1 = wload(wpool, "w_g1T", eng=sync)
                wp = wload(wpool, "w_pT", eng=sync)
                fusedT = [None, None]
                for b in range(BPC):
                    fusedT[b] = smC.tile(
                        [128, KC, NP], BF16, tag=f"fusedT{b}", bufs=1, name="fusedT"
                    )
                    for mo in range(KC):
                        ps = ppC.tile([128, 2, 512], F32, tag="pgate", bufs=4)
                        for j in range(2):
                            for k in range(KC):
                                pe.matmul(
                                    ps[:, j, :JC],
                                    wg1[:, k, mo * 128 : (mo + 1) * 128],
                                    soT[:, b, k, j * JC : (j + 1) * JC],
                                    start=(k == 0),
                                    stop=(k == KC - 1),
                                )
                        gateT = smC.tile(
                            [128, NP], BF16, tag="gateT", bufs=2, name="gateT"
                        )
                        act.activation(
                            gateT.rearrange("p (j f) -> p j f", j=2),
                            ps[:, :, :JC],
                            AF.Sigmoid,
                            bias=rowtc[:, mo, b : b + 1],
                        )
                        vec.scalar_tensor_tensor(
                            out=fusedT[b][:, mo, :],
                            in0=soT[:, b, mo, :],
                            scalar=bioc[:, mo, b : b + 1],
                            in1=gateT,
                            op0=ALU.subtract,
                            op1=ALU.mult,
                        )
                        vec.tensor_tensor(
                            out=fusedT[b][:, mo, :],
                            in0=soT[:, b, mo, :],
                            in1=fusedT[b][:, mo, :],
                            op=ALU.subtract,
                        )
                for b in range(BPC):
                    for mo in range(KC):
                        ps = ppC.tile([128, 2, 512], F32, tag="pgate", bufs=4)
                        for j in range(2):
                            for k in range(KC):
                                pe.matmul(
                                    ps[:, j, :JC],
                                    wp[:, k, mo * 128 : (mo + 1) * 128],
                                    fusedT[b][:, k, j * JC : (j + 1) * JC],
                                    start=(k == 0),
                                    stop=(k == KC - 1),
                                )
                        outT = smC.tile([128, NP], F32, tag="outT", bufs=3, name="oT")
                        vec.tensor_scalar_add(
                            outT.rearrange("p (j f) -> p j f", j=2),
                            ps[:, :, :JC],
                            bp[:, mo : mo + 1],
                        )
                        eng = sync if mo % 2 == 0 else act
                        eng.dma_start(
                            out=out.ap()[b, mo * 128 : (mo + 1) * 128, :],
                            in_=outT[:, 0:NREAL],
                        )


# ====================== host side ======================


def stage_inputs(inputs):
    """Pure layout/dtype staging of the full inputs into 8 per-core in_maps."""
    f = np.float32
    bf = ml_dtypes.bfloat16
    x = np.asarray(inputs["x"], f)
    bio = np.asarray(inputs["bio_embed"], f)
    conf = np.asarray(inputs["confidence"], f)
    bco = np.asarray(inputs["base_coords"], f)
    offs = np.asarray(inputs["offsets"], f)

    W = {k: np.asarray(v, f) for k, v in inputs.items()}
    qkv_w = W["qkv_w"]
    qkv_b = W["qkv_b"]
    wv = qkv_w[2 * C :]
    w_vT = np.zeros((C, VW), f)
    b_vw = np.zeros((1, VW), f)
    for h in range(H):
        w_vT[:, 65 * h : 65 * h + 64] = wv[64 * h : 64 * h + 64].T
        b_vw[0, 65 * h : 65 * h + 64] = qkv_b[2 * C + 64 * h : 2 * C + 64 * h + 64]
        b_vw[0, 65 * h + 64] = 1.0

    ident = np.eye(128, dtype=f)
    gz, gy, gx = np.meshgrid(np.arange(G), np.arange(G), np.arange(G), indexing="ij")
    ngrid = -np.stack([gx.ravel(), gy.ravel(), gz.ravel()], axis=1).astype(f)
    splitT = np.zeros((2, 128), f)
    splitT[0, :64] = 1.0
    splitT[1, 64:] = 1.0

    # fold bio_query into ca_query: q_ca = Wcaq @ (Wbioq @ bio + b_bioq) + b_caq
    w_caq = W["ca_in_w"][:C]
    w_caq_eff = w_caq @ W["bio_query_w"]
    b_caq_eff = w_caq @ W["bio_query_b"] + W["ca_in_b"][:C]

    shared = {
        "w_qkT": qkv_w[: 2 * C].T,
        "w_vT": w_vT,
        "w_caqT": w_caq_eff.T,
        "w_cak": W["ca_in_w"][C : 2 * C],
        "w_cavT": W["ca_in_w"][2 * C :].T,
        "w_caoT": W["ca_out_w"].T,
        "w_dbaqT": W["dba_in_w"][:C].T,
        "w_dbakvT": (W["dba_in_w"][C:] @ W["sp_w"]).T,
        "w_dbaoT": W["dba_out_w"].T,
        "w_g1T": W["gate_w"][:, :C].T,
        "w_g2T": W["gate_w"][:, C : 2 * C].T,
        "w_g3T": W["gate_w"][:, 2 * C :].T,
        "w_pT": W["proj_w"].T,
        "splitT": splitT,
        "hsplit": splitT.T,
    }
    shared = {k: np.ascontiguousarray(v, bf) for k, v in shared.items()}
    b_vwz = np.broadcast_to(b_vw, (128, VW)).copy()
    b_vwz[97:] = 0.0

    b_stack = np.stack(
        [
            b_caq_eff,
            W["ca_in_b"][2 * C :],
            W["ca_out_b"],
            W["dba_in_b"][:C],
            W["dba_in_w"][2 * C :] @ W["sp_b"] + W["dba_in_b"][2 * C :],
            W["dba_out_b"],
            np.zeros(C, np.float32),
            W["proj_b"],
        ]
    )
    # merged [128, 1762] f32 consts: ident | ngrid | confb | bqk | bstack
    # | bvw | bvwz (confb is per-core; filled below)
    cf32 = np.zeros((128, 1762), f)
    cf32[:, 0:128] = ident
    cf32[:, 128:140] = ngrid.reshape(4, 128, 3).transpose(1, 0, 2).reshape(128, 12)
    cf32[:, 142:154] = qkv_b[: 2 * C].reshape(12, 128).T
    cf32[:, 154:202] = b_stack.reshape(8, KC, 128).transpose(2, 0, 1).reshape(128, 48)
    cf32[:, 202:982] = np.broadcast_to(b_vw, (128, VW))
    cf32[:, 982:1762] = b_vwz
    shared_f32 = {
        "b_g": np.broadcast_to(W["gate_b"].reshape(1, C), (BPC, C)),
    }
    shared.update(
        {k: np.ascontiguousarray(v, f) for k, v in shared_f32.items()}
    )
    shared["bcoordsT"] = np.ascontiguousarray(
        np.broadcast_to(bco.T[:, None, :], (3, BPC, M)).reshape(1, 3, BPC * M), f
    )

    xpad = np.zeros((B, NP, C), f)
    xpad[:, :NREAL] = x

    in_maps = []
    for c in range(NCORES):
        sl = slice(c * BPC, (c + 1) * BPC)
        m = dict(shared)
        m["xn"] = np.ascontiguousarray(xpad[sl], bf)
        m["xT"] = np.ascontiguousarray(xpad[sl].transpose(0, 2, 1), bf)
        m["bioT"] = np.ascontiguousarray(bio[sl].T, bf)
        cfc = cf32.copy()
        cfc[:, 140:142] = np.broadcast_to(conf[sl].reshape(1, BPC), (128, BPC))
        m["cf32"] = cfc
        m["confT"] = np.ascontiguousarray(conf[sl].reshape(BPC, 1), f)
        m["offsT"] = np.ascontiguousarray(
            offs[sl].transpose(2, 0, 1).reshape(1, 3, BPC * M), f
        )
        in_maps.append(m)
    return in_maps


_CACHE = {}


def get_nc():
    if "nc" not in _CACHE:
        nc = bacc.Bacc("TRN2", target_bir_lowering=False, debug=False)
        build(nc)
        _CACHE["nc"] = nc
    return _CACHE["nc"]


def _ensure_ntff_hook():
    """The agent image's antenv lacks axon_hooks; shim it so trace=True can
    reach the libaxon NTFF profiler (profiling only, test-harness use)."""
    import types

    try:
        import antenv.axon_hooks  # noqa: F401

        return
    except ImportError:
        pass
    mod = types.ModuleType("antenv.axon_hooks")
    state = {"h": None}
    mod.set_axon_ntff_profile_hook = lambda h: state.__setitem__("h", h)
    mod.get_axon_ntff_profile_hook = lambda: state["h"]
    sys.modules["antenv.axon_hooks"] = mod
    import antenv

    antenv.axon_hooks = mod
    try:
        from trn_agent_boot.trn_boot import _ntff_profile_via_ctypes

        hook = _ntff_profile_via_ctypes("/opt/axon/libaxon_pjrt.so")
        if hook is not None:
            mod.set_axon_ntff_profile_hook(hook)
    except Exception:
        pass


def kernel(**inputs):
    trace = bool(int(os.environ.get("KERNEL_TRACE", "0")))
    if trace:
        _ensure_ntff_hook()
    nc = get_nc()
    in_maps = stage_inputs(inputs)
    res = bass_utils.run_bass_kernel_spmd(
        nc,
        in_maps,
        core_ids=list(range(NCORES)),
        trace=trace,
    )
    _CACHE["last_result"] = res
    outT = np.stack([res.results[c]["outT"] for c in range(NCORES)])
    out = outT.reshape(B, C, N).transpose(0, 2, 1)
    return np.ascontiguousarray(out, dtype=np.float32)
